# revision 21
# baseline (speedup 1.0000x reference)
"""GPT-style dense transformer (B=8, T=1024, D=768, H=12, L=6) on 8 NeuronCores.

Sharding: pure data parallelism over batch — each core runs one full sequence,
no collectives. On-device layout is feature-major ([D, T]) so every matmul
consumes activations directly as the moving operand; weights are the stationary
operand. Matmul operands are bf16 (fp32 PSUM accumulation); the residual
stream, layernorm statistics and softmax normalization stay fp32.

Causal masking uses one extra PE matmul per diagonal block over the 128-wide
triangle:  mask[s,j] = sum_c U[c,s] * Vr[c,j] = -1e9 * max(0, s - j),
which drives exp() to exact 0 for masked positions. Off-triangle masked
columns are simply never computed (S blocks are emitted at their visible
width). Softmax denominators come from a ones-column appended to V (M=65 AV
matmuls); normalization happens via reciprocal + rank-1 broadcast matmul
before the output projection.
"""

import numpy as np

V, D, H, HS, L, NCLS = 50257, 768, 12, 64, 6, 10
B, T = 8, 1024
FF = 4 * D
DC = D // 128          # 6 d-chunks
FC = FF // 128         # 24 ffn chunks
TT = T // 128          # 8 token tiles
EPS = 1e-5
BIG = -1.0e9

_CACHE = {}


def _build():
    import concourse.bacc as bacc
    import concourse.bass as bass
    import concourse.mybir as mybir
    import concourse.tile as tile

    F32 = mybir.dt.float32
    F32R = mybir.dt.float32r
    BF16 = mybir.dt.bfloat16
    I32 = mybir.dt.int32
    AF = mybir.ActivationFunctionType
    OP = mybir.AluOpType

    nc = bacc.Bacc(None, dynamic_dma_scratch_size=8192)

    # ---- DRAM tensors ----
    ids_d = nc.dram_tensor("ids", [T], I32, kind="ExternalInput")
    tok_d = nc.dram_tensor("tok_emb", [V, D], F32, kind="ExternalInput")
    posT_d = nc.dram_tensor("posT", [D, T], F32, kind="ExternalInput")

    # packed weights: one [128, X] contiguous block per DMA tile
    wqkv_d = nc.dram_tensor("wqkv", [L, 12, 128, D], BF16, kind="ExternalInput")
    wv_d = nc.dram_tensor("wv", [L, 2, 128, DC * 384], BF16, kind="ExternalInput")
    wo_d = nc.dram_tensor("wo", [L, DC, 128, D], BF16, kind="ExternalInput")
    w1_d = nc.dram_tensor("w1", [L, FC, 128, D], BF16, kind="ExternalInput")
    w2_d = nc.dram_tensor("w2", [L, DC, 128, FC * 128], BF16, kind="ExternalInput")
    wc_d = nc.dram_tensor("wc", [D, NCLS], BF16, kind="ExternalInput")

    qkb_d = nc.dram_tensor("qkb", [L, 128, 12], F32, kind="ExternalInput")
    b1c_d = nc.dram_tensor("b1c", [L, 128, FC], F32, kind="ExternalInput")
    borow_d = nc.dram_tensor("borow", [L, 1, D], F32, kind="ExternalInput")
    b2row_d = nc.dram_tensor("b2row", [L, 1, D], F32, kind="ExternalInput")
    bcrow_d = nc.dram_tensor("bcrow", [1, NCLS], BF16, kind="ExternalInput")


    ident_d = nc.dram_tensor("ident", [128, 128], F32, kind="ExternalInput")
    ustair_d = nc.dram_tensor("ustair", [128, 128], BF16, kind="ExternalInput")
    vramp_d = nc.dram_tensor("vramp", [128, 128], BF16, kind="ExternalInput")
    onescol_d = nc.dram_tensor("onescol", [1, 128], BF16, kind="ExternalInput")
    onesrow_d = nc.dram_tensor("onesrow", [1, 512], BF16, kind="ExternalInput")
    ones128f_d = nc.dram_tensor("ones128f", [128, 1], mybir.dt.float32r, kind="ExternalInput")
    ones128b_d = nc.dram_tensor("ones128b", [128, 1], BF16, kind="ExternalInput")
    onesv_d = nc.dram_tensor("onesv", [128, H], BF16, kind="ExternalInput")

    out_d = nc.dram_tensor("out", [1, NCLS], F32, kind="ExternalOutput")

    with tile.TileContext(nc) as tc:
        with (
            tc.tile_pool(name="res", bufs=1) as res,
            tc.tile_pool(name="wts", bufs=1) as wts,
            tc.tile_pool(name="scr", bufs=1) as scr,
            tc.tile_pool(name="ps", bufs=1, space="PSUM") as ps,
        ):
            # --- constants ---
            ident = res.tile([128, 128], F32, tag="ident")
            nc.sync.dma_start(ident[:], ident_d[:])
            ustair = res.tile([128, 128], BF16, tag="ustair")
            nc.sync.dma_start(ustair[:], ustair_d[:])
            vramp = res.tile([128, 128], BF16, tag="vramp")
            nc.sync.dma_start(vramp[:], vramp_d[:])
            onescol = res.tile([1, 128], BF16, tag="onescol")
            nc.sync.dma_start(onescol[:], onescol_d[:])
            onesrow = res.tile([1, 512], BF16, tag="onesrow")
            nc.sync.dma_start(onesrow[:], onesrow_d[:])
            ones128f = res.tile([128, 1], F32R, tag="ones128f")
            nc.sync.dma_start(ones128f[:], ones128f_d[:])
            ones128b = res.tile([128, 1], BF16, tag="ones128b")
            nc.sync.dma_start(ones128b[:], ones128b_d[:])
            epst = res.tile([128, 1], F32, tag="epst")
            nc.vector.memset(epst[:], EPS)

            # --- persistent activation tensors ---
            x = res.tile([128, DC, T], F32R, tag="x")          # residual, feature-major
            QT = res.tile([128, DC, T], BF16, tag="QT")
            KT = res.tile([128, DC, T], BF16, tag="KT")
            Vt = [res.tile([128, H, HS + 1], BF16, tag=f"V{t}", name=f"Vt{t}")
                  for t in range(TT)]

            # V ones-column (written once; V evacs only touch [:, :, 0:HS])
            for t in range(TT):
                nc.sync.dma_start(Vt[t][:, :, HS:HS + 1], onesv_d[:].unsqueeze(2))

            # ---------- embedding ----------
            idt = scr.tile([128, TT], I32, tag="ids")
            nc.sync.dma_start(idt[:], ids_d[:].rearrange("(tt p) -> p tt", p=128))
            for t in range(TT):
                xtok = scr.tile([128, D], F32, tag="xtok", bufs=4)
                nc.gpsimd.indirect_dma_start(
                    out=xtok[:], out_offset=None, in_=tok_d[:],
                    in_offset=bass.IndirectOffsetOnAxis(ap=idt[:, t:t + 1], axis=0))
                for dc in range(DC):
                    pt = ps.tile([128, 128], F32, tag="acc", bufs=2)
                    nc.tensor.transpose(pt[:], xtok[:, dc * 128:(dc + 1) * 128], ident[:])
                    pos = scr.tile([128, 128], F32, tag="pos", bufs=2)
                    nc.sync.dma_start(
                        pos[:], posT_d[dc * 128:(dc + 1) * 128, t * 128:(t + 1) * 128])
                    nc.vector.tensor_add(x[:, dc, t * 128:(t + 1) * 128], pt[:], pos[:])

            # ---------- layernorm helper ----------
            def ln_pass(lni, out_t):
                """out_t[:, dc, :] = (x - mean) * rstd  (bf16; g/b folded into
                consumer weights host-side)."""
                for J in range(2):
                    sl = slice(J * 512, (J + 1) * 512)
                    psx = ps.tile([1, 512], F32, tag="pss2", bufs=2)
                    for dc in range(DC):
                        nc.tensor.matmul(psx[:], ones128f[:], x[:, dc, sl],
                                         start=(dc == 0), stop=(dc == DC - 1))
                    psq = ps.tile([1, 512], F32, tag="pss2", bufs=2)
                    for dc in range(DC):
                        xsq = scr.tile([128, 512], BF16, tag="w512b", bufs=3)
                        nc.vector.tensor_mul(xsq[:], x[:, dc, sl], x[:, dc, sl])
                        nc.tensor.matmul(psq[:], ones128b[:], xsq[:],
                                         start=(dc == 0), stop=(dc == DC - 1))
                    sxr = scr.tile([1, 512], F32, tag="statJ", bufs=4)
                    nc.scalar.activation(sxr[:], psx[:], AF.Identity, scale=1.0 / D)
                    sq1 = scr.tile([1, 512], F32, tag="statJ", bufs=4)
                    nc.vector.tensor_mul(sq1[:], sxr[:], sxr[:])
                    svar = scr.tile([1, 512], F32, tag="statJ", bufs=4)
                    with nc.allow_low_precision(reason="LN variance combine"):
                        nc.vector.scalar_tensor_tensor(
                            out=svar[:], in0=psq[:], scalar=1.0 / D, in1=sq1[:],
                            op0=OP.mult, op1=OP.subtract)
                    lnv1 = scr.tile([1, 512], F32, tag="statJ", bufs=4)
                    nc.scalar.activation(lnv1[:], svar[:], AF.Ln, bias=epst[:1, :1])
                    rstd1 = scr.tile([1, 512], F32, tag="statJ", bufs=4)
                    nc.scalar.activation(rstd1[:], lnv1[:], AF.Exp, scale=-0.5)
                    mb = scr.tile([128, 512], F32, tag="w512", bufs=6)
                    nc.gpsimd.partition_broadcast(mb[:], sxr[:])
                    rstdb = scr.tile([128, 512], F32, tag="w512", bufs=6)
                    nc.gpsimd.partition_broadcast(rstdb[:], rstd1[:])
                    for dc in range(DC):
                        eng = nc.vector if dc % 2 == 0 else nc.gpsimd
                        t1 = scr.tile([128, 512], F32, tag="w512", bufs=6)
                        eng.tensor_tensor(out=t1[:], in0=x[:, dc, sl], in1=mb[:],
                                          op=OP.subtract)
                        eng.tensor_tensor(out=out_t[:, dc, sl], in0=t1[:],
                                          in1=rstdb[:], op=OP.mult)

            # ---------- layers ----------
            for l in range(L):
                hO = res.tile([128, DC, T], BF16, tag="hO", bufs=1, name=f"h1_{l}")
                ln_pass(2 * l, hO)

                qkb = scr.tile([128, 12], F32, tag="qkb", bufs=2)
                nc.sync.dma_start(qkb[:], qkb_d[l])

                # === Q, K projections (m 0..5 -> Q, 6..11 -> K) ===
                for m in [0, 6, 1, 7, 2, 8, 3, 9, 4, 10, 5, 11]:
                    wm = wts.tile([128, DC, 128], BF16, tag="wm", bufs=3)
                    nc.sync.dma_start(
                        wm[:], wqkv_d[l, m].rearrange("p (dc e) -> p dc e", dc=DC))
                    for J in range(2):
                        sl = slice(J * 512, (J + 1) * 512)
                        pq = ps.tile([128, 512], F32, tag="aux", bufs=2)
                        for dc in range(DC):
                            nc.tensor.matmul(pq[:], wm[:, dc, :], hO[:, dc, sl],
                                             start=(dc == 0), stop=(dc == DC - 1))
                        dst = QT if m < 6 else KT
                        nc.scalar.activation(dst[:, m % 6, sl], pq[:], AF.Identity,
                                             bias=qkb[:, m:m + 1])

                # === V projection (token-major, t-outer) ===
                vws = []
                for e2 in range(2):
                    vw = wts.tile([128, DC, 384], BF16, tag="vw", bufs=2,
                                  name=f"vw{e2}")
                    nc.sync.dma_start(
                        vw[:], wv_d[l, e2].rearrange("p (dc e) -> p dc e", dc=DC))
                    vws.append(vw)
                for t in range(TT):
                    for e2 in range(2):
                        pv = ps.tile([128, 384], F32, tag="acc", bufs=2)
                        for dc in range(DC):
                            nc.tensor.matmul(pv[:], hO[:, dc, t * 128:(t + 1) * 128],
                                             vws[e2][:, dc, :],
                                             start=(dc == 0), stop=(dc == DC - 1))
                        h0 = e2 * 6
                        nc.scalar.activation(
                            Vt[t][:, h0:h0 + 6, 0:HS],
                            pv[:].rearrange("p (h e) -> p h e", h=6), AF.Identity)

                # === attention: head pairs share one 2-bank psum, single exp ===
                def emit_S(hp, J):
                    ec = hp
                    nblk = 4 * J + 4
                    tiles = []   # per i: (at [128,2,512], c0)
                    for i in range(nblk):
                        r = i - 4 * J
                        c0 = max(r, 0) * 128
                        pss = ps.tile([128, 1024], F32, tag="pss2", bufs=2)
                        for sub in range(2):
                            po = sub * 64
                            o0 = sub * 512
                            nc.tensor.matmul(
                                pss[:, o0 + c0:o0 + 512],
                                KT[po:po + 64, ec, i * 128:(i + 1) * 128],
                                QT[po:po + 64, ec, J * 512 + c0:(J + 1) * 512],
                                start=True, stop=(r < 0))
                            if r >= 0:
                                nc.tensor.matmul(
                                    pss[:, o0 + c0:o0 + c0 + 128],
                                    ustair[:], vramp[:], start=False, stop=True)
                        at = scr.tile([128, 2, 512], BF16, tag="A", bufs=16)
                        nc.scalar.activation(
                            at[:, :, c0:512],
                            pss[:].rearrange("p (s c) -> p s c", s=2)[:, :, c0:512],
                            AF.Exp, scale=1.0 / 8)
                        tiles.append((at, c0))
                    return tiles

                def emit_AV(hp, J, tiles):
                    ec = hp
                    nblk = 4 * J + 4
                    for sub in range(2):
                        hh = 2 * hp + sub
                        po = sub * 64
                        pso = ps.tile([65, 512], F32, tag="acc", bufs=2)
                        for i in range(nblk):
                            at, c0 = tiles[i]
                            nc.tensor.matmul(pso[:, c0:512], Vt[i][:, hh, :],
                                             at[:, sub, c0:512],
                                             start=(i == 0), stop=(i == nblk - 1))
                        rec = scr.tile([1, 512], F32, tag="rec", bufs=2)
                        with nc.allow_low_precision(reason="softmax denom recip"):
                            nc.vector.reciprocal(rec[:], pso[64:65, :])
                        rb = scr.tile([64, 512], F32, tag="rb", bufs=2)
                        nc.gpsimd.partition_broadcast(rb[:], rec[:])
                        nc.vector.tensor_mul(hO[po:po + 64, ec, J * 512:(J + 1) * 512],
                                             pso[0:64, :], rb[:])

                groups = [(hp, J) for J in range(2) for hp in range(6)]

                def emit_AV_block(hp, J, tiles, psos, i):
                    nblk = 4 * J + 4
                    for sub in range(2):
                        hh = 2 * hp + sub
                        at, c0 = tiles[i]
                        nc.tensor.matmul(psos[sub][:, c0:512], Vt[i][:, hh, :],
                                         at[:, sub, c0:512],
                                         start=(i == 0), stop=(i == nblk - 1))

                def emit_AV_tail(hp, J, psos):
                    ec = hp
                    for sub in range(2):
                        po = sub * 64
                        rec = scr.tile([1, 512], F32, tag="rec", bufs=2)
                        with nc.allow_low_precision(reason="softmax denom recip"):
                            nc.vector.reciprocal(rec[:], psos[sub][64:65, :])
                        rb = scr.tile([64, 512], F32, tag="rb", bufs=2)
                        nc.gpsimd.partition_broadcast(rb[:], rec[:])
                        nc.vector.tensor_mul(hO[po:po + 64, ec, J * 512:(J + 1) * 512],
                                             psos[sub][0:64, :], rb[:])

                def emit_S_block(hp, J, i):
                    ec = hp
                    r = i - 4 * J
                    c0 = max(r, 0) * 128
                    pss = ps.tile([128, 1024], F32, tag="pss2", bufs=2)
                    for sub in range(2):
                        po = sub * 64
                        o0 = sub * 512
                        nc.tensor.matmul(
                            pss[:, o0 + c0:o0 + 512],
                            KT[po:po + 64, ec, i * 128:(i + 1) * 128],
                            QT[po:po + 64, ec, J * 512 + c0:(J + 1) * 512],
                            start=True, stop=(r < 0))
                        if r >= 0:
                            nc.tensor.matmul(
                                pss[:, o0 + c0:o0 + c0 + 128],
                                ustair[:], vramp[:], start=False, stop=True)
                    at = scr.tile([128, 2, 512], BF16, tag="A", bufs=16)
                    nc.scalar.activation(
                        at[:, :, c0:512],
                        pss[:].rearrange("p (s c) -> p s c", s=2)[:, :, c0:512],
                        AF.Exp, scale=1.0 / 8)
                    return (at, c0)

                pend = None   # (hp, J, tiles, psos)
                for (hp, J) in groups:
                    nblk = 4 * J + 4
                    pblk = (4 * pend[1] + 4) if pend else 0
                    tiles = [None] * nblk
                    for i in range(max(nblk, pblk)):
                        if i < nblk:
                            tiles[i] = emit_S_block(hp, J, i)
                        if pend and i < pblk:
                            emit_AV_block(pend[0], pend[1], pend[2], pend[3], i)
                    if pend:
                        emit_AV_tail(pend[0], pend[1], pend[3])
                    psos = [ps.tile([65, 512], F32, tag="acc", bufs=2,
                                    name=f"pso{sub}") for sub in range(2)]
                    pend = (hp, J, tiles, psos)
                for i in range(4 * pend[1] + 4):
                    emit_AV_block(pend[0], pend[1], pend[2], pend[3], i)
                emit_AV_tail(pend[0], pend[1], pend[3])

                # === output projection + residual (bias fused into DVE) ===
                bocol = scr.tile([128, DC], F32, tag="bocol", bufs=2)
                nc.sync.dma_start(bocol[:], borow_d[l].rearrange(
                    "one (dc p) -> p (one dc)", p=128))
                for m in range(DC):
                    wot = wts.tile([128, DC, 128], BF16, tag="wm", bufs=3)
                    nc.sync.dma_start(
                        wot[:], wo_d[l, m].rearrange("p (dc e) -> p dc e", dc=DC))
                    for J in range(2):
                        sl = slice(J * 512, (J + 1) * 512)
                        p2 = ps.tile([128, 512], F32, tag="aux", bufs=2)
                        for dc in range(DC):
                            nc.tensor.matmul(p2[:], wot[:, dc, :], hO[:, dc, sl],
                                             start=(dc == 0), stop=(dc == DC - 1))
                        nc.vector.scalar_tensor_tensor(
                            out=x[:, m, sl], in0=p2[:], scalar=bocol[:, m:m + 1],
                            in1=x[:, m, sl], op0=OP.add, op1=OP.add)

                # === LN2 -> h2 ===
                h2 = res.tile([128, DC, T], BF16, tag="hO", bufs=1, name=f"h2_{l}")
                ln_pass(2 * l + 1, h2)

                # === FFN: FFN1 streams, FFN2 m-outer over resident gf/w2 ===
                b1c = scr.tile([128, FC], F32, tag="b1c", bufs=2)
                nc.sync.dma_start(b1c[:], b1c_d[l])
                b2col = scr.tile([128, DC], F32, tag="b2col", bufs=2)
                nc.sync.dma_start(b2col[:], b2row_d[l].rearrange(
                    "one (dc p) -> p (one dc)", p=128))
                for J in range(2):
                    sl = slice(J * 512, (J + 1) * 512)
                    gfs = []
                    for fc in range(FC):
                        w1t = wts.tile([128, DC, 128], BF16, tag="w1", bufs=3)
                        nc.sync.dma_start(
                            w1t[:], w1_d[l, fc].rearrange("p (dc e) -> p dc e", dc=DC))
                        pf = ps.tile([128, 512], F32, tag="aux", bufs=2)
                        for dc in range(DC):
                            nc.tensor.matmul(pf[:], w1t[:, dc, :], h2[:, dc, sl],
                                             start=(dc == 0), stop=(dc == DC - 1))
                        gf = scr.tile([128, 512], BF16, tag="gf", bufs=FC + 2)
                        nc.scalar.activation(gf[:], pf[:], AF.Gelu,
                                             bias=b1c[:, fc:fc + 1])
                        gfs.append(gf)
                    for m in range(DC):
                        w2m = wts.tile([128, FC, 128], BF16, tag="w2m", bufs=3)
                        nc.scalar.dma_start(
                            w2m[:], w2_d[l, m].rearrange("p (fc e) -> p fc e", fc=FC))
                        p2 = ps.tile([128, 512], F32, tag="acc", bufs=2)
                        for fc in range(FC):
                            nc.tensor.matmul(p2[:], w2m[:, fc, :],
                                             gfs[fc][:], start=(fc == 0),
                                             stop=(fc == FC - 1))
                        nc.vector.scalar_tensor_tensor(
                            out=x[:, m, sl], in0=p2[:], scalar=b2col[:, m:m + 1],
                            in1=x[:, m, sl], op0=OP.add, op1=OP.add)

            # ---------- final LN + mean pool + classifier ----------
            hf = res.tile([128, DC, T], BF16, tag="hO", bufs=1, name="hf")
            ln_pass(2 * L, hf)
            wct = scr.tile([128, DC, NCLS], BF16, tag="wct")
            nc.sync.dma_start(wct[:], wc_d[:].rearrange("(dc p) n -> p dc n", p=128))
            bcr = scr.tile([1, NCLS], BF16, tag="bcr")
            nc.sync.dma_start(bcr[:], bcrow_d[:])
            pooled = []
            for dc in range(DC):
                red = scr.tile([128, 1], F32, tag="red", bufs=DC)
                nc.vector.tensor_reduce(red[:], hf[:, dc, :], axis=mybir.AxisListType.X,
                                        op=OP.add)
                pr = scr.tile([128, 1], BF16, tag="pooled", bufs=DC)
                nc.scalar.activation(pr[:], red[:], AF.Identity, scale=1.0 / T)
                pooled.append(pr)
            pcls = ps.tile([1, NCLS], F32, tag="acc", bufs=2)
            nc.tensor.matmul(pcls[:], onescol[:, :1], bcr[:], start=True, stop=False)
            for dc in range(DC):
                nc.tensor.matmul(pcls[:], pooled[dc][:], wct[:, dc, :],
                                 start=False, stop=(dc == DC - 1))
            ob = scr.tile([1, NCLS], F32, tag="ob")
            nc.vector.tensor_copy(ob[:], pcls[:])
            nc.sync.dma_start(out_d[:], ob[:])

    nc.compile()
    return nc


def _pack_params(params):
    import ml_dtypes

    BF = ml_dtypes.bfloat16
    g = {k: np.asarray(v, dtype=np.float32) for k, v in params.items()}

    # ---- fold LN gains/biases into consumer projections ----
    # device LN emits (x-m)*rstd; true h = g*that + b, so for consumer
    # y = W^T h + c:  W' = diag(g) @ W,  c' = c + W^T b.
    wq = g["wq"].transpose(0, 2, 1, 3).reshape(L, D, D)
    wk = g["wk"].transpose(0, 2, 1, 3).reshape(L, D, D)
    wv = g["wv"].transpose(0, 2, 1, 3).reshape(L, D, D)
    bq = g["bq"].reshape(L, D).copy()
    bk = g["bk"].reshape(L, D).copy()
    bv = g["bv"].reshape(L, D).copy()
    w1 = g["w1"].copy()
    b1 = g["b1"].copy()
    wc = g["wc"].copy()
    bc = g["bc"].copy()
    bo = g["bo"].copy()
    wo = g["wo"]

    for l in range(L):
        g1, bql = g["ln1_g"][l], g["ln1_b"][l]
        bq[l] += wq[l].T @ bql
        bk[l] += wk[l].T @ bql
        bv[l] += wv[l].T @ bql
        wq[l] = wq[l] * g1[:, None]
        wk[l] = wk[l] * g1[:, None]
        wv[l] = wv[l] * g1[:, None]
        g2, b2l = g["ln2_g"][l], g["ln2_b"][l]
        b1[l] += w1[l].T @ b2l
        w1[l] = w1[l] * g2[:, None]
        # V bias folds through attention into the output projection bias:
        # O = raw/den + bv  =>  bo' = bo + wo^T bv
        bo[l] += wo[l].T @ bv[l]
    bc += wc.T @ g["lnf_b"]
    wc = wc * g["lnf_g"][:, None]

    wqk = np.concatenate([wq, wk], axis=2)                       # [L, D, 2D]

    # packed lhsT tiles: tile[l, m, p, dc*128+e] = W[l, dc*128+p, m*128+e]
    def pack_lhsT(w, mtiles):
        return np.ascontiguousarray(
            w.reshape(L, DC, 128, mtiles, 128).transpose(0, 3, 2, 1, 4)
            .reshape(L, mtiles, 128, DC * 128)).astype(BF)

    wqkv_p = pack_lhsT(wqk, 12)
    # V weight (moving operand): tile[l, e2, p, dc*384+e] = wv[l, dc*128+p, e2*384+e]
    wv_p = np.ascontiguousarray(
        wv.reshape(L, DC, 128, 2, 384).transpose(0, 3, 2, 1, 4)
        .reshape(L, 2, 128, DC * 384)).astype(BF)
    wo_p = pack_lhsT(wo, DC)
    w1_p = pack_lhsT(w1, FC)
    # w2 per-m lhsT tiles: w2m[l, m, p, fc*128+e] = w2[l, fc*128+p, m*128+e]
    w2_p = np.ascontiguousarray(
        g["w2"].reshape(L, FC, 128, DC, 128).transpose(0, 3, 2, 1, 4)
        .reshape(L, DC, 128, FC * 128)).astype(BF)

    qkb = np.empty((L, 128, 12), np.float32)
    for l in range(L):
        qk = np.concatenate([bq[l], bk[l]])
        for m in range(12):
            qkb[l, :, m] = qk[m * 128:(m + 1) * 128]

    b1c = np.empty((L, 128, FC), np.float32)
    for l in range(L):
        for fc in range(FC):
            b1c[l, :, fc] = b1[l][fc * 128:(fc + 1) * 128]

    # causal-mask generators over the diagonal 128-block triangle:
    # mask[s, j] = sum_c U[c,s]*Vr[c,j] = BIG * max(0, s - j)
    cc = np.arange(128)
    ustair = (cc[None, :] >= cc[:, None]).astype(BF)
    vramp = np.where(cc[None, :] < cc[:, None], np.float32(BIG), 0.0).astype(BF)

    return {
        "tok_emb": g["tok_emb"],
        "posT": np.ascontiguousarray(g["pos_emb"][:T].T),
        "wqkv": wqkv_p,
        "wv": wv_p,
        "wo": wo_p,
        "w1": w1_p,
        "w2": w2_p,
        "wc": wc.astype(BF),
        "qkb": qkb,
        "b1c": b1c,
        "borow": bo.reshape(L, 1, D),
        "b2row": g["b2"].reshape(L, 1, D),
        "bcrow": bc.reshape(1, NCLS).astype(BF),
        "ident": np.eye(128, dtype=np.float32),
        "ustair": ustair,
        "vramp": vramp,
        "onescol": np.ones((1, 128), BF),
        "onesrow": np.ones((1, 512), BF),
        "ones128f": np.ones((128, 1), np.float32),
        "ones128b": np.ones((128, 1), BF),
        "onesv": np.ones((128, H), BF),
    }


def kernel(token_ids, params):
    from concourse.bass_utils import run_bass_kernel_spmd

    if "nc" not in _CACHE:
        _CACHE["nc"] = _build()
    nc = _CACHE["nc"]

    common = _pack_params(params)
    ids32 = np.asarray(token_ids).astype(np.int32)
    in_maps = [{**common, "ids": ids32[c]} for c in range(B)]
    res = run_bass_kernel_spmd(nc, in_maps, list(range(B)))
    out = np.stack([res.results[c]["out"][0] for c in range(B)], axis=0)
    return out.astype(np.float32)


# revision 22
# speedup vs baseline: 1.1298x; 1.1298x over previous
"""GPT-style dense transformer (B=8, T=1024, D=768, H=12, L=6) on 8 NeuronCores.

Sharding: pure data parallelism over batch — each core runs one full sequence,
no collectives. On-device layout is feature-major ([D, T]) so every matmul
consumes activations directly as the moving operand; weights are the stationary
operand. Matmul operands are bf16 (fp32 PSUM accumulation); the residual
stream, layernorm statistics and softmax normalization stay fp32.

Causal masking uses one extra PE matmul per diagonal block over the 128-wide
triangle:  mask[s,j] = sum_c U[c,s] * Vr[c,j] = -1e9 * max(0, s - j),
which drives exp() to exact 0 for masked positions. Off-triangle masked
columns are simply never computed (S blocks are emitted at their visible
width). Softmax denominators come from a ones-column appended to V (M=65 AV
matmuls); normalization happens via reciprocal + rank-1 broadcast matmul
before the output projection.
"""

import numpy as np

V, D, H, HS, L, NCLS = 50257, 768, 12, 64, 6, 10
B, T = 8, 1024
FF = 4 * D
DC = D // 128          # 6 d-chunks
FC = FF // 128         # 24 ffn chunks
TT = T // 128          # 8 token tiles
EPS = 1e-5
BIG = -1.0e9

_CACHE = {}


def _build():
    import concourse.bacc as bacc
    import concourse.bass as bass
    import concourse.mybir as mybir
    import concourse.tile as tile

    F32 = mybir.dt.float32
    F32R = mybir.dt.float32r
    BF16 = mybir.dt.bfloat16
    I32 = mybir.dt.int32
    AF = mybir.ActivationFunctionType
    OP = mybir.AluOpType

    nc = bacc.Bacc(None, dynamic_dma_scratch_size=8192)

    # ---- DRAM tensors ----
    ids_d = nc.dram_tensor("ids", [T], I32, kind="ExternalInput")
    tok_d = nc.dram_tensor("tok_emb", [V, D], F32, kind="ExternalInput")
    posT_d = nc.dram_tensor("posT", [D, T], F32, kind="ExternalInput")

    # packed weights: one [128, X] contiguous block per DMA tile
    wqkv_d = nc.dram_tensor("wqkv", [L, 12, 128, D], BF16, kind="ExternalInput")
    wv_d = nc.dram_tensor("wv", [L, 2, 128, DC * 384], BF16, kind="ExternalInput")
    wo_d = nc.dram_tensor("wo", [L, DC, 128, D], BF16, kind="ExternalInput")
    w1_d = nc.dram_tensor("w1", [L, FC, 128, D], BF16, kind="ExternalInput")
    w2_d = nc.dram_tensor("w2", [L, DC, 128, FC * 128], BF16, kind="ExternalInput")
    wc_d = nc.dram_tensor("wc", [D, NCLS], BF16, kind="ExternalInput")

    qkb_d = nc.dram_tensor("qkb", [L, 128, 12], F32, kind="ExternalInput")
    b1c_d = nc.dram_tensor("b1c", [L, 128, FC], F32, kind="ExternalInput")
    borow_d = nc.dram_tensor("borow", [L, 1, D], F32, kind="ExternalInput")
    b2row_d = nc.dram_tensor("b2row", [L, 1, D], F32, kind="ExternalInput")
    bcrow_d = nc.dram_tensor("bcrow", [1, NCLS], BF16, kind="ExternalInput")


    ident_d = nc.dram_tensor("ident", [128, 128], F32, kind="ExternalInput")
    ustair_d = nc.dram_tensor("ustair", [128, 128], BF16, kind="ExternalInput")
    vramp_d = nc.dram_tensor("vramp", [128, 128], BF16, kind="ExternalInput")
    onescol_d = nc.dram_tensor("onescol", [1, 128], BF16, kind="ExternalInput")
    onesrow_d = nc.dram_tensor("onesrow", [1, 512], BF16, kind="ExternalInput")
    ones128f_d = nc.dram_tensor("ones128f", [128, 1], mybir.dt.float32r, kind="ExternalInput")
    ones128b_d = nc.dram_tensor("ones128b", [128, 1], BF16, kind="ExternalInput")
    onesv_d = nc.dram_tensor("onesv", [128, H], BF16, kind="ExternalInput")

    out_d = nc.dram_tensor("out", [1, NCLS], F32, kind="ExternalOutput")

    with tile.TileContext(nc) as tc:
        with (
            tc.tile_pool(name="res", bufs=1) as res,
            tc.tile_pool(name="wts", bufs=1) as wts,
            tc.tile_pool(name="scr", bufs=1) as scr,
            tc.tile_pool(name="ps", bufs=1, space="PSUM") as ps,
        ):
            # --- constants ---
            ident = res.tile([128, 128], F32, tag="ident")
            nc.sync.dma_start(ident[:], ident_d[:])
            ustair = res.tile([128, 128], BF16, tag="ustair")
            nc.sync.dma_start(ustair[:], ustair_d[:])
            vramp = res.tile([128, 128], BF16, tag="vramp")
            nc.sync.dma_start(vramp[:], vramp_d[:])
            onescol = res.tile([1, 128], BF16, tag="onescol")
            nc.sync.dma_start(onescol[:], onescol_d[:])
            onesrow = res.tile([1, 512], BF16, tag="onesrow")
            nc.sync.dma_start(onesrow[:], onesrow_d[:])
            ones128f = res.tile([128, 1], F32R, tag="ones128f")
            nc.sync.dma_start(ones128f[:], ones128f_d[:])
            ones128b = res.tile([128, 1], BF16, tag="ones128b")
            nc.sync.dma_start(ones128b[:], ones128b_d[:])
            epst = res.tile([128, 1], F32, tag="epst")
            nc.vector.memset(epst[:], EPS)

            # --- persistent activation tensors ---
            x = res.tile([128, DC, T], F32R, tag="x")          # residual, feature-major
            QT = res.tile([128, DC, T], BF16, tag="QT")
            KT = res.tile([128, DC, T], BF16, tag="KT")
            Vt = [res.tile([128, H, HS + 1], BF16, tag=f"V{t}", name=f"Vt{t}")
                  for t in range(TT)]

            # V ones-column (written once; V evacs only touch [:, :, 0:HS])
            for t in range(TT):
                nc.sync.dma_start(Vt[t][:, :, HS:HS + 1], onesv_d[:].unsqueeze(2))

            # ---------- embedding ----------
            idt = scr.tile([128, TT], I32, tag="ids")
            nc.sync.dma_start(idt[:], ids_d[:].rearrange("(tt p) -> p tt", p=128))
            for t in range(TT):
                xtok = scr.tile([128, D], F32, tag="xtok", bufs=4)
                nc.gpsimd.indirect_dma_start(
                    out=xtok[:], out_offset=None, in_=tok_d[:],
                    in_offset=bass.IndirectOffsetOnAxis(ap=idt[:, t:t + 1], axis=0))
                for dc in range(DC):
                    pt = ps.tile([128, 128], F32, tag="acc", bufs=2)
                    nc.tensor.transpose(pt[:], xtok[:, dc * 128:(dc + 1) * 128], ident[:])
                    pos = scr.tile([128, 128], F32, tag="pos", bufs=2)
                    nc.sync.dma_start(
                        pos[:], posT_d[dc * 128:(dc + 1) * 128, t * 128:(t + 1) * 128])
                    nc.vector.tensor_add(x[:, dc, t * 128:(t + 1) * 128], pt[:], pos[:])

            # ---------- layernorm helper ----------
            def ln_pass(lni, out_t):
                """out_t[:, dc, :] = (x - mean) * rstd  (bf16; g/b folded into
                consumer weights host-side)."""
                for J in range(2):
                    sl = slice(J * 512, (J + 1) * 512)
                    psx = ps.tile([1, 512], F32, tag="pss2", bufs=2)
                    for dc in range(DC):
                        nc.tensor.matmul(psx[:], ones128f[:], x[:, dc, sl],
                                         start=(dc == 0), stop=(dc == DC - 1))
                    psq = ps.tile([1, 512], F32, tag="pss2", bufs=2)
                    for dc in range(DC):
                        xsq = scr.tile([128, 512], BF16, tag="w512b", bufs=3)
                        nc.vector.tensor_mul(xsq[:], x[:, dc, sl], x[:, dc, sl])
                        nc.tensor.matmul(psq[:], ones128b[:], xsq[:],
                                         start=(dc == 0), stop=(dc == DC - 1))
                    sxr = scr.tile([1, 512], F32, tag="statJ", bufs=4)
                    nc.scalar.activation(sxr[:], psx[:], AF.Identity, scale=1.0 / D)
                    sq1 = scr.tile([1, 512], F32, tag="statJ", bufs=4)
                    nc.vector.tensor_mul(sq1[:], sxr[:], sxr[:])
                    svar = scr.tile([1, 512], F32, tag="statJ", bufs=4)
                    with nc.allow_low_precision(reason="LN variance combine"):
                        nc.vector.scalar_tensor_tensor(
                            out=svar[:], in0=psq[:], scalar=1.0 / D, in1=sq1[:],
                            op0=OP.mult, op1=OP.subtract)
                    lnv1 = scr.tile([1, 512], F32, tag="statJ", bufs=4)
                    nc.scalar.activation(lnv1[:], svar[:], AF.Ln, bias=epst[:1, :1])
                    rstd1 = scr.tile([1, 512], F32, tag="statJ", bufs=4)
                    nc.scalar.activation(rstd1[:], lnv1[:], AF.Exp, scale=-0.5)
                    mb = scr.tile([128, 512], F32, tag="w512", bufs=6)
                    nc.gpsimd.partition_broadcast(mb[:], sxr[:])
                    rstdb = scr.tile([128, 512], F32, tag="w512", bufs=6)
                    nc.gpsimd.partition_broadcast(rstdb[:], rstd1[:])
                    for dc in range(DC):
                        t1 = scr.tile([128, 512], F32, tag="w512", bufs=6)
                        nc.vector.tensor_tensor(out=t1[:], in0=x[:, dc, sl], in1=mb[:],
                                                op=OP.subtract)
                        nc.vector.tensor_mul(out_t[:, dc, sl], t1[:], rstdb[:])

            # ---------- layers ----------
            for l in range(L):
                hO = res.tile([128, DC, T], BF16, tag="hO", bufs=1, name=f"h1_{l}")
                ln_pass(2 * l, hO)

                qkb = scr.tile([128, 12], F32, tag="qkb", bufs=2)
                nc.sync.dma_start(qkb[:], qkb_d[l])

                # === Q, K projections (m 0..5 -> Q, 6..11 -> K) ===
                for m in [0, 6, 1, 7, 2, 8, 3, 9, 4, 10, 5, 11]:
                    wm = wts.tile([128, DC, 128], BF16, tag="wm", bufs=3)
                    nc.sync.dma_start(
                        wm[:], wqkv_d[l, m].rearrange("p (dc e) -> p dc e", dc=DC))
                    for J in range(2):
                        sl = slice(J * 512, (J + 1) * 512)
                        pq = ps.tile([128, 512], F32, tag="aux", bufs=2)
                        for dc in range(DC):
                            nc.tensor.matmul(pq[:], wm[:, dc, :], hO[:, dc, sl],
                                             start=(dc == 0), stop=(dc == DC - 1))
                        dst = QT if m < 6 else KT
                        nc.scalar.activation(dst[:, m % 6, sl], pq[:], AF.Identity,
                                             bias=qkb[:, m:m + 1])

                # === V projection (token-major, t-outer) ===
                vws = []
                for e2 in range(2):
                    vw = wts.tile([128, DC, 384], BF16, tag="vw", bufs=2,
                                  name=f"vw{e2}")
                    nc.sync.dma_start(
                        vw[:], wv_d[l, e2].rearrange("p (dc e) -> p dc e", dc=DC))
                    vws.append(vw)
                for t in range(TT):
                    for e2 in range(2):
                        pv = ps.tile([128, 384], F32, tag="acc", bufs=2)
                        for dc in range(DC):
                            nc.tensor.matmul(pv[:], hO[:, dc, t * 128:(t + 1) * 128],
                                             vws[e2][:, dc, :],
                                             start=(dc == 0), stop=(dc == DC - 1))
                        h0 = e2 * 6
                        nc.scalar.activation(
                            Vt[t][:, h0:h0 + 6, 0:HS],
                            pv[:].rearrange("p (h e) -> p h e", h=6), AF.Identity)

                # === attention: head pairs share one 2-bank psum, single exp ===
                def emit_S(hp, J):
                    ec = hp
                    nblk = 4 * J + 4
                    tiles = []   # per i: (at [128,2,512], c0)
                    for i in range(nblk):
                        r = i - 4 * J
                        c0 = max(r, 0) * 128
                        pss = ps.tile([128, 1024], F32, tag="pss2", bufs=2)
                        for sub in range(2):
                            po = sub * 64
                            o0 = sub * 512
                            nc.tensor.matmul(
                                pss[:, o0 + c0:o0 + 512],
                                KT[po:po + 64, ec, i * 128:(i + 1) * 128],
                                QT[po:po + 64, ec, J * 512 + c0:(J + 1) * 512],
                                start=True, stop=(r < 0))
                            if r >= 0:
                                nc.tensor.matmul(
                                    pss[:, o0 + c0:o0 + c0 + 128],
                                    ustair[:], vramp[:], start=False, stop=True)
                        at = scr.tile([128, 2, 512], BF16, tag="A", bufs=16)
                        nc.scalar.activation(
                            at[:, :, c0:512],
                            pss[:].rearrange("p (s c) -> p s c", s=2)[:, :, c0:512],
                            AF.Exp, scale=1.0 / 8)
                        tiles.append((at, c0))
                    return tiles

                def emit_AV(hp, J, tiles):
                    ec = hp
                    nblk = 4 * J + 4
                    for sub in range(2):
                        hh = 2 * hp + sub
                        po = sub * 64
                        pso = ps.tile([65, 512], F32, tag="acc", bufs=2)
                        for i in range(nblk):
                            at, c0 = tiles[i]
                            nc.tensor.matmul(pso[:, c0:512], Vt[i][:, hh, :],
                                             at[:, sub, c0:512],
                                             start=(i == 0), stop=(i == nblk - 1))
                        rec = scr.tile([1, 512], F32, tag="rec", bufs=2)
                        with nc.allow_low_precision(reason="softmax denom recip"):
                            nc.vector.reciprocal(rec[:], pso[64:65, :])
                        rb = scr.tile([64, 512], F32, tag="rb", bufs=2)
                        nc.gpsimd.partition_broadcast(rb[:], rec[:])
                        nc.vector.tensor_mul(hO[po:po + 64, ec, J * 512:(J + 1) * 512],
                                             pso[0:64, :], rb[:])

                groups = [(hp, J) for J in range(2) for hp in range(6)]

                def emit_AV_block(hp, J, tiles, psos, i):
                    nblk = 4 * J + 4
                    for sub in range(2):
                        hh = 2 * hp + sub
                        at, c0 = tiles[i]
                        nc.tensor.matmul(psos[sub][:, c0:512], Vt[i][:, hh, :],
                                         at[:, sub, c0:512],
                                         start=(i == 0), stop=(i == nblk - 1))

                def emit_AV_tail(hp, J, psos):
                    ec = hp
                    for sub in range(2):
                        po = sub * 64
                        rec = scr.tile([1, 512], F32, tag="rec", bufs=2)
                        with nc.allow_low_precision(reason="softmax denom recip"):
                            nc.vector.reciprocal(rec[:], psos[sub][64:65, :])
                        rb = scr.tile([64, 512], F32, tag="rb", bufs=2)
                        nc.gpsimd.partition_broadcast(rb[:], rec[:])
                        nc.vector.tensor_mul(hO[po:po + 64, ec, J * 512:(J + 1) * 512],
                                             psos[sub][0:64, :], rb[:])

                def emit_S_block(hp, J, i):
                    ec = hp
                    r = i - 4 * J
                    c0 = max(r, 0) * 128
                    pss = ps.tile([128, 1024], F32, tag="pss2", bufs=2)
                    for sub in range(2):
                        po = sub * 64
                        o0 = sub * 512
                        nc.tensor.matmul(
                            pss[:, o0 + c0:o0 + 512],
                            KT[po:po + 64, ec, i * 128:(i + 1) * 128],
                            QT[po:po + 64, ec, J * 512 + c0:(J + 1) * 512],
                            start=True, stop=(r < 0))
                        if r >= 0:
                            nc.tensor.matmul(
                                pss[:, o0 + c0:o0 + c0 + 128],
                                ustair[:], vramp[:], start=False, stop=True)
                    at = scr.tile([128, 2, 512], BF16, tag="A", bufs=16)
                    nc.scalar.activation(
                        at[:, :, c0:512],
                        pss[:].rearrange("p (s c) -> p s c", s=2)[:, :, c0:512],
                        AF.Exp, scale=1.0 / 8)
                    return (at, c0)

                pend = None   # (hp, J, tiles, psos)
                for (hp, J) in groups:
                    nblk = 4 * J + 4
                    pblk = (4 * pend[1] + 4) if pend else 0
                    tiles = [None] * nblk
                    for i in range(max(nblk, pblk)):
                        if i < nblk:
                            tiles[i] = emit_S_block(hp, J, i)
                        if pend and i < pblk:
                            emit_AV_block(pend[0], pend[1], pend[2], pend[3], i)
                    if pend:
                        emit_AV_tail(pend[0], pend[1], pend[3])
                    psos = [ps.tile([65, 512], F32, tag="acc", bufs=2,
                                    name=f"pso{sub}") for sub in range(2)]
                    pend = (hp, J, tiles, psos)
                for i in range(4 * pend[1] + 4):
                    emit_AV_block(pend[0], pend[1], pend[2], pend[3], i)
                emit_AV_tail(pend[0], pend[1], pend[3])

                # === output projection + residual (bias fused into DVE) ===
                bocol = scr.tile([128, DC], F32, tag="bocol", bufs=2)
                nc.sync.dma_start(bocol[:], borow_d[l].rearrange(
                    "one (dc p) -> p (one dc)", p=128))
                for m in range(DC):
                    wot = wts.tile([128, DC, 128], BF16, tag="wm", bufs=3)
                    nc.sync.dma_start(
                        wot[:], wo_d[l, m].rearrange("p (dc e) -> p dc e", dc=DC))
                    for J in range(2):
                        sl = slice(J * 512, (J + 1) * 512)
                        p2 = ps.tile([128, 512], F32, tag="aux", bufs=2)
                        for dc in range(DC):
                            nc.tensor.matmul(p2[:], wot[:, dc, :], hO[:, dc, sl],
                                             start=(dc == 0), stop=(dc == DC - 1))
                        nc.vector.scalar_tensor_tensor(
                            out=x[:, m, sl], in0=p2[:], scalar=bocol[:, m:m + 1],
                            in1=x[:, m, sl], op0=OP.add, op1=OP.add)

                # === LN2 -> h2 ===
                h2 = res.tile([128, DC, T], BF16, tag="hO", bufs=1, name=f"h2_{l}")
                ln_pass(2 * l + 1, h2)

                # === FFN: FFN1 streams, FFN2 m-outer over resident gf/w2 ===
                b1c = scr.tile([128, FC], F32, tag="b1c", bufs=2)
                nc.sync.dma_start(b1c[:], b1c_d[l])
                b2col = scr.tile([128, DC], F32, tag="b2col", bufs=2)
                nc.sync.dma_start(b2col[:], b2row_d[l].rearrange(
                    "one (dc p) -> p (one dc)", p=128))
                for J in range(2):
                    sl = slice(J * 512, (J + 1) * 512)
                    gfs = []
                    for fc in range(FC):
                        w1t = wts.tile([128, DC, 128], BF16, tag="w1", bufs=3)
                        nc.sync.dma_start(
                            w1t[:], w1_d[l, fc].rearrange("p (dc e) -> p dc e", dc=DC))
                        pf = ps.tile([128, 512], F32, tag="aux", bufs=2)
                        for dc in range(DC):
                            nc.tensor.matmul(pf[:], w1t[:, dc, :], h2[:, dc, sl],
                                             start=(dc == 0), stop=(dc == DC - 1))
                        gf = scr.tile([128, 512], BF16, tag="gf", bufs=FC + 2)
                        nc.scalar.activation(gf[:], pf[:], AF.Gelu,
                                             bias=b1c[:, fc:fc + 1])
                        gfs.append(gf)
                    for m in range(DC):
                        w2m = wts.tile([128, FC, 128], BF16, tag="w2m", bufs=3)
                        nc.scalar.dma_start(
                            w2m[:], w2_d[l, m].rearrange("p (fc e) -> p fc e", fc=FC))
                        p2 = ps.tile([128, 512], F32, tag="acc", bufs=2)
                        for fc in range(FC):
                            nc.tensor.matmul(p2[:], w2m[:, fc, :],
                                             gfs[fc][:], start=(fc == 0),
                                             stop=(fc == FC - 1))
                        nc.vector.scalar_tensor_tensor(
                            out=x[:, m, sl], in0=p2[:], scalar=b2col[:, m:m + 1],
                            in1=x[:, m, sl], op0=OP.add, op1=OP.add)

            # ---------- final LN + mean pool + classifier ----------
            hf = res.tile([128, DC, T], BF16, tag="hO", bufs=1, name="hf")
            ln_pass(2 * L, hf)
            wct = scr.tile([128, DC, NCLS], BF16, tag="wct")
            nc.sync.dma_start(wct[:], wc_d[:].rearrange("(dc p) n -> p dc n", p=128))
            bcr = scr.tile([1, NCLS], BF16, tag="bcr")
            nc.sync.dma_start(bcr[:], bcrow_d[:])
            pooled = []
            for dc in range(DC):
                red = scr.tile([128, 1], F32, tag="red", bufs=DC)
                nc.vector.tensor_reduce(red[:], hf[:, dc, :], axis=mybir.AxisListType.X,
                                        op=OP.add)
                pr = scr.tile([128, 1], BF16, tag="pooled", bufs=DC)
                nc.scalar.activation(pr[:], red[:], AF.Identity, scale=1.0 / T)
                pooled.append(pr)
            pcls = ps.tile([1, NCLS], F32, tag="acc", bufs=2)
            nc.tensor.matmul(pcls[:], onescol[:, :1], bcr[:], start=True, stop=False)
            for dc in range(DC):
                nc.tensor.matmul(pcls[:], pooled[dc][:], wct[:, dc, :],
                                 start=False, stop=(dc == DC - 1))
            ob = scr.tile([1, NCLS], F32, tag="ob")
            nc.vector.tensor_copy(ob[:], pcls[:])
            nc.sync.dma_start(out_d[:], ob[:])

    nc.compile()
    return nc


def _pack_params(params):
    import ml_dtypes

    BF = ml_dtypes.bfloat16
    g = {k: np.asarray(v, dtype=np.float32) for k, v in params.items()}

    # ---- fold LN gains/biases into consumer projections ----
    # device LN emits (x-m)*rstd; true h = g*that + b, so for consumer
    # y = W^T h + c:  W' = diag(g) @ W,  c' = c + W^T b.
    wq = g["wq"].transpose(0, 2, 1, 3).reshape(L, D, D)
    wk = g["wk"].transpose(0, 2, 1, 3).reshape(L, D, D)
    wv = g["wv"].transpose(0, 2, 1, 3).reshape(L, D, D)
    bq = g["bq"].reshape(L, D).copy()
    bk = g["bk"].reshape(L, D).copy()
    bv = g["bv"].reshape(L, D).copy()
    w1 = g["w1"].copy()
    b1 = g["b1"].copy()
    wc = g["wc"].copy()
    bc = g["bc"].copy()
    bo = g["bo"].copy()
    wo = g["wo"]

    for l in range(L):
        g1, bql = g["ln1_g"][l], g["ln1_b"][l]
        bq[l] += wq[l].T @ bql
        bk[l] += wk[l].T @ bql
        bv[l] += wv[l].T @ bql
        wq[l] = wq[l] * g1[:, None]
        wk[l] = wk[l] * g1[:, None]
        wv[l] = wv[l] * g1[:, None]
        g2, b2l = g["ln2_g"][l], g["ln2_b"][l]
        b1[l] += w1[l].T @ b2l
        w1[l] = w1[l] * g2[:, None]
        # V bias folds through attention into the output projection bias:
        # O = raw/den + bv  =>  bo' = bo + wo^T bv
        bo[l] += wo[l].T @ bv[l]
    bc += wc.T @ g["lnf_b"]
    wc = wc * g["lnf_g"][:, None]

    wqk = np.concatenate([wq, wk], axis=2)                       # [L, D, 2D]

    # packed lhsT tiles: tile[l, m, p, dc*128+e] = W[l, dc*128+p, m*128+e]
    def pack_lhsT(w, mtiles):
        return np.ascontiguousarray(
            w.reshape(L, DC, 128, mtiles, 128).transpose(0, 3, 2, 1, 4)
            .reshape(L, mtiles, 128, DC * 128)).astype(BF)

    wqkv_p = pack_lhsT(wqk, 12)
    # V weight (moving operand): tile[l, e2, p, dc*384+e] = wv[l, dc*128+p, e2*384+e]
    wv_p = np.ascontiguousarray(
        wv.reshape(L, DC, 128, 2, 384).transpose(0, 3, 2, 1, 4)
        .reshape(L, 2, 128, DC * 384)).astype(BF)
    wo_p = pack_lhsT(wo, DC)
    w1_p = pack_lhsT(w1, FC)
    # w2 per-m lhsT tiles: w2m[l, m, p, fc*128+e] = w2[l, fc*128+p, m*128+e]
    w2_p = np.ascontiguousarray(
        g["w2"].reshape(L, FC, 128, DC, 128).transpose(0, 3, 2, 1, 4)
        .reshape(L, DC, 128, FC * 128)).astype(BF)

    qkb = np.empty((L, 128, 12), np.float32)
    for l in range(L):
        qk = np.concatenate([bq[l], bk[l]])
        for m in range(12):
            qkb[l, :, m] = qk[m * 128:(m + 1) * 128]

    b1c = np.empty((L, 128, FC), np.float32)
    for l in range(L):
        for fc in range(FC):
            b1c[l, :, fc] = b1[l][fc * 128:(fc + 1) * 128]

    # causal-mask generators over the diagonal 128-block triangle:
    # mask[s, j] = sum_c U[c,s]*Vr[c,j] = BIG * max(0, s - j)
    cc = np.arange(128)
    ustair = (cc[None, :] >= cc[:, None]).astype(BF)
    vramp = np.where(cc[None, :] < cc[:, None], np.float32(BIG), 0.0).astype(BF)

    return {
        "tok_emb": g["tok_emb"],
        "posT": np.ascontiguousarray(g["pos_emb"][:T].T),
        "wqkv": wqkv_p,
        "wv": wv_p,
        "wo": wo_p,
        "w1": w1_p,
        "w2": w2_p,
        "wc": wc.astype(BF),
        "qkb": qkb,
        "b1c": b1c,
        "borow": bo.reshape(L, 1, D),
        "b2row": g["b2"].reshape(L, 1, D),
        "bcrow": bc.reshape(1, NCLS).astype(BF),
        "ident": np.eye(128, dtype=np.float32),
        "ustair": ustair,
        "vramp": vramp,
        "onescol": np.ones((1, 128), BF),
        "onesrow": np.ones((1, 512), BF),
        "ones128f": np.ones((128, 1), np.float32),
        "ones128b": np.ones((128, 1), BF),
        "onesv": np.ones((128, H), BF),
    }


def kernel(token_ids, params):
    from concourse.bass_utils import run_bass_kernel_spmd

    if "nc" not in _CACHE:
        _CACHE["nc"] = _build()
    nc = _CACHE["nc"]

    common = _pack_params(params)
    ids32 = np.asarray(token_ids).astype(np.int32)
    in_maps = [{**common, "ids": ids32[c]} for c in range(B)]
    res = run_bass_kernel_spmd(nc, in_maps, list(range(B)))
    out = np.stack([res.results[c]["out"][0] for c in range(B)], axis=0)
    return out.astype(np.float32)


# revision 23
# speedup vs baseline: 1.1640x; 1.0303x over previous
"""GPT-style dense transformer (B=8, T=1024, D=768, H=12, L=6) on 8 NeuronCores.

Sharding: pure data parallelism over batch — each core runs one full sequence,
no collectives. On-device layout is feature-major ([D, T]) so every matmul
consumes activations directly as the moving operand; weights are the stationary
operand. Matmul operands are bf16 (fp32 PSUM accumulation); the residual
stream, layernorm statistics and softmax normalization stay fp32.

Causal masking uses one extra PE matmul per diagonal block over the 128-wide
triangle:  mask[s,j] = sum_c U[c,s] * Vr[c,j] = -1e9 * max(0, s - j),
which drives exp() to exact 0 for masked positions. Off-triangle masked
columns are simply never computed (S blocks are emitted at their visible
width). Softmax denominators come from a ones-column appended to V (M=65 AV
matmuls); normalization happens via reciprocal + rank-1 broadcast matmul
before the output projection.
"""

import numpy as np

V, D, H, HS, L, NCLS = 50257, 768, 12, 64, 6, 10
B, T = 8, 1024
FF = 4 * D
DC = D // 128          # 6 d-chunks
FC = FF // 128         # 24 ffn chunks
TT = T // 128          # 8 token tiles
EPS = 1e-5
BIG = -1.0e9

_CACHE = {}


def _build():
    import concourse.bacc as bacc
    import concourse.bass as bass
    import concourse.mybir as mybir
    import concourse.tile as tile

    F32 = mybir.dt.float32
    F32R = mybir.dt.float32r
    BF16 = mybir.dt.bfloat16
    I32 = mybir.dt.int32
    AF = mybir.ActivationFunctionType
    OP = mybir.AluOpType

    nc = bacc.Bacc(None, dynamic_dma_scratch_size=8192)

    # ---- DRAM tensors ----
    ids_d = nc.dram_tensor("ids", [T], I32, kind="ExternalInput")
    tok_d = nc.dram_tensor("tok_emb", [V, D], F32, kind="ExternalInput")
    posT_d = nc.dram_tensor("posT", [D, T], F32, kind="ExternalInput")

    # packed weights: one [128, X] contiguous block per DMA tile
    wqkv_d = nc.dram_tensor("wqkv", [L, 12, 128, D], BF16, kind="ExternalInput")
    wv_d = nc.dram_tensor("wv", [L, 2, 128, DC * 384], BF16, kind="ExternalInput")
    wo_d = nc.dram_tensor("wo", [L, DC, 128, D], BF16, kind="ExternalInput")
    w1_d = nc.dram_tensor("w1", [L, FC, 128, D], BF16, kind="ExternalInput")
    w2_d = nc.dram_tensor("w2", [L, DC, 128, FC * 128], BF16, kind="ExternalInput")
    wc_d = nc.dram_tensor("wc", [D, NCLS], BF16, kind="ExternalInput")

    qkb_d = nc.dram_tensor("qkb", [L, 128, 12], F32, kind="ExternalInput")
    b1c_d = nc.dram_tensor("b1c", [L, 128, FC], F32, kind="ExternalInput")
    borow_d = nc.dram_tensor("borow", [L, 1, D], F32, kind="ExternalInput")
    b2row_d = nc.dram_tensor("b2row", [L, 1, D], F32, kind="ExternalInput")
    bcrow_d = nc.dram_tensor("bcrow", [1, NCLS], BF16, kind="ExternalInput")


    ident_d = nc.dram_tensor("ident", [128, 128], F32, kind="ExternalInput")
    ustair_d = nc.dram_tensor("ustair", [128, 128], BF16, kind="ExternalInput")
    vramp_d = nc.dram_tensor("vramp", [128, 128], BF16, kind="ExternalInput")
    onescol_d = nc.dram_tensor("onescol", [1, 128], BF16, kind="ExternalInput")
    onesrow_d = nc.dram_tensor("onesrow", [1, 512], BF16, kind="ExternalInput")
    ones128f_d = nc.dram_tensor("ones128f", [128, 1], mybir.dt.float32r, kind="ExternalInput")
    ones128b_d = nc.dram_tensor("ones128b", [128, 1], BF16, kind="ExternalInput")
    onesv_d = nc.dram_tensor("onesv", [128, H], BF16, kind="ExternalInput")

    out_d = nc.dram_tensor("out", [1, NCLS], F32, kind="ExternalOutput")

    with tile.TileContext(nc) as tc:
        with (
            tc.tile_pool(name="res", bufs=1) as res,
            tc.tile_pool(name="wts", bufs=1) as wts,
            tc.tile_pool(name="scr", bufs=1) as scr,
            tc.tile_pool(name="ps", bufs=1, space="PSUM") as ps,
        ):
            # --- constants ---
            ident = res.tile([128, 128], F32, tag="ident")
            nc.sync.dma_start(ident[:], ident_d[:])
            ustair = res.tile([128, 128], BF16, tag="ustair")
            nc.sync.dma_start(ustair[:], ustair_d[:])
            vramp = res.tile([128, 128], BF16, tag="vramp")
            nc.sync.dma_start(vramp[:], vramp_d[:])
            onescol = res.tile([1, 128], BF16, tag="onescol")
            nc.sync.dma_start(onescol[:], onescol_d[:])
            onesrow = res.tile([1, 512], BF16, tag="onesrow")
            nc.sync.dma_start(onesrow[:], onesrow_d[:])
            ones128f = res.tile([128, 1], F32R, tag="ones128f")
            nc.sync.dma_start(ones128f[:], ones128f_d[:])
            ones128b = res.tile([128, 1], BF16, tag="ones128b")
            nc.sync.dma_start(ones128b[:], ones128b_d[:])
            epst = res.tile([128, 1], F32, tag="epst")
            nc.vector.memset(epst[:], EPS)

            # --- persistent activation tensors ---
            x = res.tile([128, DC, T], F32R, tag="x")          # residual, feature-major
            QT = res.tile([128, DC, T], BF16, tag="QT")
            KT = res.tile([128, DC, T], BF16, tag="KT")
            Vt = [res.tile([128, H, HS + 1], BF16, tag=f"V{t}", name=f"Vt{t}")
                  for t in range(TT)]

            # V ones-column (written once; V evacs only touch [:, :, 0:HS])
            for t in range(TT):
                nc.sync.dma_start(Vt[t][:, :, HS:HS + 1], onesv_d[:].unsqueeze(2))

            # ---------- embedding ----------
            idt = scr.tile([128, TT], I32, tag="ids")
            nc.sync.dma_start(idt[:], ids_d[:].rearrange("(tt p) -> p tt", p=128))
            for t in range(TT):
                xtok = scr.tile([128, D], F32, tag="xtok", bufs=4)
                nc.gpsimd.indirect_dma_start(
                    out=xtok[:], out_offset=None, in_=tok_d[:],
                    in_offset=bass.IndirectOffsetOnAxis(ap=idt[:, t:t + 1], axis=0))
                for dc in range(DC):
                    pt = ps.tile([128, 128], F32, tag="acc", bufs=2)
                    nc.tensor.transpose(pt[:], xtok[:, dc * 128:(dc + 1) * 128], ident[:])
                    pos = scr.tile([128, 128], F32, tag="pos", bufs=2)
                    nc.sync.dma_start(
                        pos[:], posT_d[dc * 128:(dc + 1) * 128, t * 128:(t + 1) * 128])
                    nc.vector.tensor_add(x[:, dc, t * 128:(t + 1) * 128], pt[:], pos[:])

            # ---------- layernorm helper ----------
            def ln_pass(lni, out_t):
                """out_t[:, dc, :] = (x - mean) * rstd  (bf16; g/b folded into
                consumer weights host-side)."""
                for J in range(2):
                    sl = slice(J * 512, (J + 1) * 512)
                    psx = ps.tile([1, 512], F32, tag="pss2", bufs=2)
                    for dc in range(DC):
                        nc.tensor.matmul(psx[:], ones128f[:], x[:, dc, sl],
                                         start=(dc == 0), stop=(dc == DC - 1))
                    psq = ps.tile([1, 512], F32, tag="pss2", bufs=2)
                    for dc in range(DC):
                        xsq = scr.tile([128, 512], BF16, tag="w512b", bufs=3)
                        nc.vector.tensor_mul(xsq[:], x[:, dc, sl], x[:, dc, sl])
                        nc.tensor.matmul(psq[:], ones128b[:], xsq[:],
                                         start=(dc == 0), stop=(dc == DC - 1))
                    sxr = scr.tile([1, 512], F32, tag="statJ", bufs=4)
                    nc.scalar.activation(sxr[:], psx[:], AF.Identity, scale=1.0 / D)
                    sq1 = scr.tile([1, 512], F32, tag="statJ", bufs=4)
                    nc.vector.tensor_mul(sq1[:], sxr[:], sxr[:])
                    svar = scr.tile([1, 512], F32, tag="statJ", bufs=4)
                    with nc.allow_low_precision(reason="LN variance combine"):
                        nc.vector.scalar_tensor_tensor(
                            out=svar[:], in0=psq[:], scalar=1.0 / D, in1=sq1[:],
                            op0=OP.mult, op1=OP.subtract)
                    lnv1 = scr.tile([1, 512], F32, tag="statJ", bufs=4)
                    nc.scalar.activation(lnv1[:], svar[:], AF.Ln, bias=epst[:1, :1])
                    rstd1 = scr.tile([1, 512], F32, tag="statJ", bufs=4)
                    nc.scalar.activation(rstd1[:], lnv1[:], AF.Exp, scale=-0.5)
                    mb = scr.tile([128, 512], F32, tag="w512", bufs=6)
                    nc.gpsimd.partition_broadcast(mb[:], sxr[:])
                    rstdb = scr.tile([128, 512], F32, tag="w512", bufs=6)
                    nc.gpsimd.partition_broadcast(rstdb[:], rstd1[:])
                    for dc in range(DC):
                        t1 = scr.tile([128, 512], F32, tag="w512", bufs=6)
                        nc.vector.tensor_tensor(out=t1[:], in0=x[:, dc, sl], in1=mb[:],
                                                op=OP.subtract)
                        nc.vector.tensor_mul(out_t[:, dc, sl], t1[:], rstdb[:])

            # ---------- layers ----------
            for l in range(L):
                hO = res.tile([128, DC, T], BF16, tag="hO", bufs=1, name=f"h1_{l}")
                ln_pass(2 * l, hO)

                qkb = scr.tile([128, 12], F32, tag="qkb", bufs=2)
                nc.sync.dma_start(qkb[:], qkb_d[l])

                # === Q, K projections (m 0..5 -> Q, 6..11 -> K) ===
                for m in [0, 6, 1, 7, 2, 8, 3, 9, 4, 10, 5, 11]:
                    wm = wts.tile([128, DC, 128], BF16, tag="wm", bufs=3)
                    nc.sync.dma_start(
                        wm[:], wqkv_d[l, m].rearrange("p (dc e) -> p dc e", dc=DC))
                    for J in range(2):
                        sl = slice(J * 512, (J + 1) * 512)
                        pq = ps.tile([128, 512], F32, tag="aux", bufs=2)
                        for dc in range(DC):
                            nc.tensor.matmul(pq[:], wm[:, dc, :], hO[:, dc, sl],
                                             start=(dc == 0), stop=(dc == DC - 1))
                        dst = QT if m < 6 else KT
                        nc.scalar.activation(dst[:, m % 6, sl], pq[:], AF.Identity,
                                             bias=qkb[:, m:m + 1])

                # === V projection (token-major, t-outer) ===
                vws = []
                for e2 in range(2):
                    vw = wts.tile([128, DC, 384], BF16, tag="vw", bufs=2,
                                  name=f"vw{e2}")
                    nc.sync.dma_start(
                        vw[:], wv_d[l, e2].rearrange("p (dc e) -> p dc e", dc=DC))
                    vws.append(vw)
                for t in range(TT):
                    for e2 in range(2):
                        pv = ps.tile([128, 384], F32, tag="acc", bufs=2)
                        for dc in range(DC):
                            nc.tensor.matmul(pv[:], hO[:, dc, t * 128:(t + 1) * 128],
                                             vws[e2][:, dc, :],
                                             start=(dc == 0), stop=(dc == DC - 1))
                        h0 = e2 * 6
                        nc.scalar.activation(
                            Vt[t][:, h0:h0 + 6, 0:HS],
                            pv[:].rearrange("p (h e) -> p h e", h=6), AF.Identity)

                # === attention: head pairs share one 2-bank psum, single exp ===
                def emit_S(hp, J):
                    ec = hp
                    nblk = 4 * J + 4
                    tiles = []   # per i: (at [128,2,512], c0)
                    for i in range(nblk):
                        r = i - 4 * J
                        c0 = max(r, 0) * 128
                        pss = ps.tile([128, 1024], F32, tag="pss2", bufs=2)
                        for sub in range(2):
                            po = sub * 64
                            o0 = sub * 512
                            nc.tensor.matmul(
                                pss[:, o0 + c0:o0 + 512],
                                KT[po:po + 64, ec, i * 128:(i + 1) * 128],
                                QT[po:po + 64, ec, J * 512 + c0:(J + 1) * 512],
                                start=True, stop=(r < 0))
                            if r >= 0:
                                nc.tensor.matmul(
                                    pss[:, o0 + c0:o0 + c0 + 128],
                                    ustair[:], vramp[:], start=False, stop=True)
                        at = scr.tile([128, 2, 512], BF16, tag="A", bufs=16)
                        nc.scalar.activation(
                            at[:, :, c0:512],
                            pss[:].rearrange("p (s c) -> p s c", s=2)[:, :, c0:512],
                            AF.Exp, scale=1.0 / 8)
                        tiles.append((at, c0))
                    return tiles

                def emit_AV(hp, J, tiles):
                    ec = hp
                    nblk = 4 * J + 4
                    for sub in range(2):
                        hh = 2 * hp + sub
                        po = sub * 64
                        pso = ps.tile([65, 512], F32, tag="acc", bufs=2)
                        for i in range(nblk):
                            at, c0 = tiles[i]
                            nc.tensor.matmul(pso[:, c0:512], Vt[i][:, hh, :],
                                             at[:, sub, c0:512],
                                             start=(i == 0), stop=(i == nblk - 1))
                        rec = scr.tile([1, 512], F32, tag="rec", bufs=2)
                        with nc.allow_low_precision(reason="softmax denom recip"):
                            nc.vector.reciprocal(rec[:], pso[64:65, :])
                        rb = scr.tile([64, 512], F32, tag="rb", bufs=2)
                        nc.gpsimd.partition_broadcast(rb[:], rec[:])
                        nc.vector.tensor_mul(hO[po:po + 64, ec, J * 512:(J + 1) * 512],
                                             pso[0:64, :], rb[:])

                # === attention: head pairs share one 2-bank psum, single exp ===
                def emit_S(hp, J):
                    ec = hp
                    nblk = 4 * J + 4
                    tiles = []
                    for i in range(nblk):
                        r = i - 4 * J
                        c0 = max(r, 0) * 128
                        pss = ps.tile([128, 1024], F32, tag="pss2", bufs=2)
                        for sub in range(2):
                            po = sub * 64
                            o0 = sub * 512
                            nc.tensor.matmul(
                                pss[:, o0 + c0:o0 + 512],
                                KT[po:po + 64, ec, i * 128:(i + 1) * 128],
                                QT[po:po + 64, ec, J * 512 + c0:(J + 1) * 512],
                                start=True, stop=(r < 0))
                            if r >= 0:
                                nc.tensor.matmul(
                                    pss[:, o0 + c0:o0 + c0 + 128],
                                    ustair[:], vramp[:], start=False, stop=True)
                        at = scr.tile([128, 2, 512], BF16, tag="A", bufs=16)
                        nc.scalar.activation(
                            at[:, :, c0:512],
                            pss[:].rearrange("p (s c) -> p s c", s=2)[:, :, c0:512],
                            AF.Exp, scale=1.0 / 8)
                        tiles.append((at, c0))
                    return tiles

                def emit_AV(hp, J, tiles):
                    ec = hp
                    nblk = 4 * J + 4
                    for sub in range(2):
                        hh = 2 * hp + sub
                        po = sub * 64
                        pso = ps.tile([65, 512], F32, tag="acc", bufs=2)
                        for i in range(nblk):
                            at, c0 = tiles[i]
                            nc.tensor.matmul(pso[:, c0:512], Vt[i][:, hh, :],
                                             at[:, sub, c0:512],
                                             start=(i == 0), stop=(i == nblk - 1))
                        rec = scr.tile([1, 512], F32, tag="rec", bufs=2)
                        with nc.allow_low_precision(reason="softmax denom recip"):
                            nc.vector.reciprocal(rec[:], pso[64:65, :])
                        rb = scr.tile([64, 512], F32, tag="rb", bufs=2)
                        nc.gpsimd.partition_broadcast(rb[:], rec[:])
                        nc.vector.tensor_mul(hO[po:po + 64, ec, J * 512:(J + 1) * 512],
                                             pso[0:64, :], rb[:])

                groups = [(hp, J) for J in range(2) for hp in range(6)]
                pend = None
                for gg in groups:
                    tiles = emit_S(*gg)
                    if pend is not None:
                        emit_AV(pend[0][0], pend[0][1], pend[1])
                    pend = (gg, tiles)
                emit_AV(pend[0][0], pend[0][1], pend[1])

                # === output projection + residual (bias fused into DVE) ===
                bocol = scr.tile([128, DC], F32, tag="bocol", bufs=2)
                nc.sync.dma_start(bocol[:], borow_d[l].rearrange(
                    "one (dc p) -> p (one dc)", p=128))
                for m in range(DC):
                    wot = wts.tile([128, DC, 128], BF16, tag="wm", bufs=3)
                    nc.sync.dma_start(
                        wot[:], wo_d[l, m].rearrange("p (dc e) -> p dc e", dc=DC))
                    for J in range(2):
                        sl = slice(J * 512, (J + 1) * 512)
                        p2 = ps.tile([128, 512], F32, tag="aux", bufs=2)
                        for dc in range(DC):
                            nc.tensor.matmul(p2[:], wot[:, dc, :], hO[:, dc, sl],
                                             start=(dc == 0), stop=(dc == DC - 1))
                        nc.vector.scalar_tensor_tensor(
                            out=x[:, m, sl], in0=p2[:], scalar=bocol[:, m:m + 1],
                            in1=x[:, m, sl], op0=OP.add, op1=OP.add)

                # === LN2 -> h2 ===
                h2 = res.tile([128, DC, T], BF16, tag="hO", bufs=1, name=f"h2_{l}")
                ln_pass(2 * l + 1, h2)

                # === FFN: FFN1 streams, FFN2 m-outer over resident gf/w2 ===
                b1c = scr.tile([128, FC], F32, tag="b1c", bufs=2)
                nc.sync.dma_start(b1c[:], b1c_d[l])
                b2col = scr.tile([128, DC], F32, tag="b2col", bufs=2)
                nc.sync.dma_start(b2col[:], b2row_d[l].rearrange(
                    "one (dc p) -> p (one dc)", p=128))
                for J in range(2):
                    sl = slice(J * 512, (J + 1) * 512)
                    gfs = []
                    for fc in range(FC):
                        w1t = wts.tile([128, DC, 128], BF16, tag="w1", bufs=3)
                        nc.sync.dma_start(
                            w1t[:], w1_d[l, fc].rearrange("p (dc e) -> p dc e", dc=DC))
                        pf = ps.tile([128, 512], F32, tag="aux", bufs=2)
                        for dc in range(DC):
                            nc.tensor.matmul(pf[:], w1t[:, dc, :], h2[:, dc, sl],
                                             start=(dc == 0), stop=(dc == DC - 1))
                        gf = scr.tile([128, 512], BF16, tag="gf", bufs=FC + 2)
                        nc.scalar.activation(gf[:], pf[:], AF.Gelu,
                                             bias=b1c[:, fc:fc + 1])
                        gfs.append(gf)
                    for m in range(DC):
                        w2m = wts.tile([128, FC, 128], BF16, tag="w2m", bufs=3)
                        nc.scalar.dma_start(
                            w2m[:], w2_d[l, m].rearrange("p (fc e) -> p fc e", fc=FC))
                        p2 = ps.tile([128, 512], F32, tag="acc", bufs=2)
                        for fc in range(FC):
                            nc.tensor.matmul(p2[:], w2m[:, fc, :],
                                             gfs[fc][:], start=(fc == 0),
                                             stop=(fc == FC - 1))
                        nc.vector.scalar_tensor_tensor(
                            out=x[:, m, sl], in0=p2[:], scalar=b2col[:, m:m + 1],
                            in1=x[:, m, sl], op0=OP.add, op1=OP.add)

            # ---------- final LN + mean pool + classifier ----------
            hf = res.tile([128, DC, T], BF16, tag="hO", bufs=1, name="hf")
            ln_pass(2 * L, hf)
            wct = scr.tile([128, DC, NCLS], BF16, tag="wct")
            nc.sync.dma_start(wct[:], wc_d[:].rearrange("(dc p) n -> p dc n", p=128))
            bcr = scr.tile([1, NCLS], BF16, tag="bcr")
            nc.sync.dma_start(bcr[:], bcrow_d[:])
            pooled = []
            for dc in range(DC):
                red = scr.tile([128, 1], F32, tag="red", bufs=DC)
                nc.vector.tensor_reduce(red[:], hf[:, dc, :], axis=mybir.AxisListType.X,
                                        op=OP.add)
                pr = scr.tile([128, 1], BF16, tag="pooled", bufs=DC)
                nc.scalar.activation(pr[:], red[:], AF.Identity, scale=1.0 / T)
                pooled.append(pr)
            pcls = ps.tile([1, NCLS], F32, tag="acc", bufs=2)
            nc.tensor.matmul(pcls[:], onescol[:, :1], bcr[:], start=True, stop=False)
            for dc in range(DC):
                nc.tensor.matmul(pcls[:], pooled[dc][:], wct[:, dc, :],
                                 start=False, stop=(dc == DC - 1))
            ob = scr.tile([1, NCLS], F32, tag="ob")
            nc.vector.tensor_copy(ob[:], pcls[:])
            nc.sync.dma_start(out_d[:], ob[:])

    nc.compile()
    return nc


def _pack_params(params):
    import ml_dtypes

    BF = ml_dtypes.bfloat16
    g = {k: np.asarray(v, dtype=np.float32) for k, v in params.items()}

    # ---- fold LN gains/biases into consumer projections ----
    # device LN emits (x-m)*rstd; true h = g*that + b, so for consumer
    # y = W^T h + c:  W' = diag(g) @ W,  c' = c + W^T b.
    wq = g["wq"].transpose(0, 2, 1, 3).reshape(L, D, D)
    wk = g["wk"].transpose(0, 2, 1, 3).reshape(L, D, D)
    wv = g["wv"].transpose(0, 2, 1, 3).reshape(L, D, D)
    bq = g["bq"].reshape(L, D).copy()
    bk = g["bk"].reshape(L, D).copy()
    bv = g["bv"].reshape(L, D).copy()
    w1 = g["w1"].copy()
    b1 = g["b1"].copy()
    wc = g["wc"].copy()
    bc = g["bc"].copy()
    bo = g["bo"].copy()
    wo = g["wo"]

    for l in range(L):
        g1, bql = g["ln1_g"][l], g["ln1_b"][l]
        bq[l] += wq[l].T @ bql
        bk[l] += wk[l].T @ bql
        bv[l] += wv[l].T @ bql
        wq[l] = wq[l] * g1[:, None]
        wk[l] = wk[l] * g1[:, None]
        wv[l] = wv[l] * g1[:, None]
        g2, b2l = g["ln2_g"][l], g["ln2_b"][l]
        b1[l] += w1[l].T @ b2l
        w1[l] = w1[l] * g2[:, None]
        # V bias folds through attention into the output projection bias:
        # O = raw/den + bv  =>  bo' = bo + wo^T bv
        bo[l] += wo[l].T @ bv[l]
    bc += wc.T @ g["lnf_b"]
    wc = wc * g["lnf_g"][:, None]

    wqk = np.concatenate([wq, wk], axis=2)                       # [L, D, 2D]

    # packed lhsT tiles: tile[l, m, p, dc*128+e] = W[l, dc*128+p, m*128+e]
    def pack_lhsT(w, mtiles):
        return np.ascontiguousarray(
            w.reshape(L, DC, 128, mtiles, 128).transpose(0, 3, 2, 1, 4)
            .reshape(L, mtiles, 128, DC * 128)).astype(BF)

    wqkv_p = pack_lhsT(wqk, 12)
    # V weight (moving operand): tile[l, e2, p, dc*384+e] = wv[l, dc*128+p, e2*384+e]
    wv_p = np.ascontiguousarray(
        wv.reshape(L, DC, 128, 2, 384).transpose(0, 3, 2, 1, 4)
        .reshape(L, 2, 128, DC * 384)).astype(BF)
    wo_p = pack_lhsT(wo, DC)
    w1_p = pack_lhsT(w1, FC)
    # w2 per-m lhsT tiles: w2m[l, m, p, fc*128+e] = w2[l, fc*128+p, m*128+e]
    w2_p = np.ascontiguousarray(
        g["w2"].reshape(L, FC, 128, DC, 128).transpose(0, 3, 2, 1, 4)
        .reshape(L, DC, 128, FC * 128)).astype(BF)

    qkb = np.empty((L, 128, 12), np.float32)
    for l in range(L):
        qk = np.concatenate([bq[l], bk[l]])
        for m in range(12):
            qkb[l, :, m] = qk[m * 128:(m + 1) * 128]

    b1c = np.empty((L, 128, FC), np.float32)
    for l in range(L):
        for fc in range(FC):
            b1c[l, :, fc] = b1[l][fc * 128:(fc + 1) * 128]

    # causal-mask generators over the diagonal 128-block triangle:
    # mask[s, j] = sum_c U[c,s]*Vr[c,j] = BIG * max(0, s - j)
    cc = np.arange(128)
    ustair = (cc[None, :] >= cc[:, None]).astype(BF)
    vramp = np.where(cc[None, :] < cc[:, None], np.float32(BIG), 0.0).astype(BF)

    return {
        "tok_emb": g["tok_emb"],
        "posT": np.ascontiguousarray(g["pos_emb"][:T].T),
        "wqkv": wqkv_p,
        "wv": wv_p,
        "wo": wo_p,
        "w1": w1_p,
        "w2": w2_p,
        "wc": wc.astype(BF),
        "qkb": qkb,
        "b1c": b1c,
        "borow": bo.reshape(L, 1, D),
        "b2row": g["b2"].reshape(L, 1, D),
        "bcrow": bc.reshape(1, NCLS).astype(BF),
        "ident": np.eye(128, dtype=np.float32),
        "ustair": ustair,
        "vramp": vramp,
        "onescol": np.ones((1, 128), BF),
        "onesrow": np.ones((1, 512), BF),
        "ones128f": np.ones((128, 1), np.float32),
        "ones128b": np.ones((128, 1), BF),
        "onesv": np.ones((128, H), BF),
    }


def kernel(token_ids, params):
    from concourse.bass_utils import run_bass_kernel_spmd

    if "nc" not in _CACHE:
        _CACHE["nc"] = _build()
    nc = _CACHE["nc"]

    common = _pack_params(params)
    ids32 = np.asarray(token_ids).astype(np.int32)
    in_maps = [{**common, "ids": ids32[c]} for c in range(B)]
    res = run_bass_kernel_spmd(nc, in_maps, list(range(B)))
    out = np.stack([res.results[c]["out"][0] for c in range(B)], axis=0)
    return out.astype(np.float32)


# revision 26
# speedup vs baseline: 1.3309x; 1.1434x over previous
"""GPT-style dense transformer (B=8, T=1024, D=768, H=12, L=6) on 8 NeuronCores.

Sharding: pure data parallelism over batch — each core runs one full sequence,
no collectives. On-device layout is feature-major ([D, T]) so every matmul
consumes activations directly as the moving operand; weights are the stationary
operand. Matmul operands are bf16 (fp32 PSUM accumulation); the residual
stream, layernorm statistics and softmax normalization stay fp32.

Causal masking uses one extra PE matmul per diagonal block over the 128-wide
triangle:  mask[s,j] = sum_c U[c,s] * Vr[c,j] = -1e9 * max(0, s - j),
which drives exp() to exact 0 for masked positions. Off-triangle masked
columns are simply never computed (S blocks are emitted at their visible
width). Softmax denominators come from a ones-column appended to V (M=65 AV
matmuls); normalization happens via reciprocal + rank-1 broadcast matmul
before the output projection.
"""

import numpy as np

V, D, H, HS, L, NCLS = 50257, 768, 12, 64, 6, 10
B, T = 8, 1024
FF = 4 * D
DC = D // 128          # 6 d-chunks
FC = FF // 128         # 24 ffn chunks
TT = T // 128          # 8 token tiles
EPS = 1e-5
BIG = -1.0e9

_CACHE = {}


def _build():
    import concourse.bacc as bacc
    import concourse.bass as bass
    import concourse.mybir as mybir
    import concourse.tile as tile

    F32 = mybir.dt.float32
    F32R = mybir.dt.float32r
    BF16 = mybir.dt.bfloat16
    I32 = mybir.dt.int32
    AF = mybir.ActivationFunctionType
    OP = mybir.AluOpType

    nc = bacc.Bacc(None, dynamic_dma_scratch_size=8192)

    # ---- DRAM tensors ----
    ids_d = nc.dram_tensor("ids", [T], I32, kind="ExternalInput")
    tok_d = nc.dram_tensor("tok_emb", [V, D], F32, kind="ExternalInput")
    posT_d = nc.dram_tensor("posT", [D, T], F32, kind="ExternalInput")

    # packed weights: one [128, X] contiguous block per DMA tile
    wqkv_d = nc.dram_tensor("wqkv", [L, 12, 128, D], BF16, kind="ExternalInput")
    wv_d = nc.dram_tensor("wv", [L, 2, 128, DC * 384], BF16, kind="ExternalInput")
    wo_d = nc.dram_tensor("wo", [L, DC, 128, D], BF16, kind="ExternalInput")
    w1_d = nc.dram_tensor("w1", [L, FC, 128, D], BF16, kind="ExternalInput")
    w2_d = nc.dram_tensor("w2", [L, DC, 128, FC * 128], BF16, kind="ExternalInput")
    wc_d = nc.dram_tensor("wc", [D, NCLS], BF16, kind="ExternalInput")

    qkb_d = nc.dram_tensor("qkb", [L, 128, 12], F32, kind="ExternalInput")
    b1c_d = nc.dram_tensor("b1c", [L, 128, FC], F32, kind="ExternalInput")
    borow_d = nc.dram_tensor("borow", [L, 1, D], F32, kind="ExternalInput")
    b2row_d = nc.dram_tensor("b2row", [L, 1, D], F32, kind="ExternalInput")
    bcrow_d = nc.dram_tensor("bcrow", [1, NCLS], BF16, kind="ExternalInput")


    ident_d = nc.dram_tensor("ident", [128, 128], F32, kind="ExternalInput")
    ustair_d = nc.dram_tensor("ustair", [128, 128], BF16, kind="ExternalInput")
    vramp_d = nc.dram_tensor("vramp", [128, 128], BF16, kind="ExternalInput")
    onescol_d = nc.dram_tensor("onescol", [1, 128], BF16, kind="ExternalInput")
    onesrow_d = nc.dram_tensor("onesrow", [1, 512], BF16, kind="ExternalInput")
    ones128f_d = nc.dram_tensor("ones128f", [128, 1], mybir.dt.float32r, kind="ExternalInput")
    ones128b_d = nc.dram_tensor("ones128b", [128, 1], BF16, kind="ExternalInput")
    onesv_d = nc.dram_tensor("onesv", [128, H], BF16, kind="ExternalInput")

    out_d = nc.dram_tensor("out", [1, NCLS], F32, kind="ExternalOutput")

    with tile.TileContext(nc) as tc:
        with (
            tc.tile_pool(name="res", bufs=1) as res,
            tc.tile_pool(name="wts", bufs=1) as wts,
            tc.tile_pool(name="scr", bufs=1) as scr,
            tc.tile_pool(name="ps", bufs=1, space="PSUM") as ps,
        ):
            # --- constants ---
            ident = res.tile([128, 128], F32, tag="ident")
            nc.sync.dma_start(ident[:], ident_d[:])
            ustair = res.tile([128, 128], BF16, tag="ustair")
            nc.sync.dma_start(ustair[:], ustair_d[:])
            vramp = res.tile([128, 128], BF16, tag="vramp")
            nc.sync.dma_start(vramp[:], vramp_d[:])
            onescol = res.tile([1, 128], BF16, tag="onescol")
            nc.sync.dma_start(onescol[:], onescol_d[:])
            onesrow = res.tile([1, 512], BF16, tag="onesrow")
            nc.sync.dma_start(onesrow[:], onesrow_d[:])
            ones128f = res.tile([128, 1], F32R, tag="ones128f")
            nc.sync.dma_start(ones128f[:], ones128f_d[:])
            ones128b = res.tile([128, 1], BF16, tag="ones128b")
            nc.sync.dma_start(ones128b[:], ones128b_d[:])
            epst = res.tile([128, 1], F32, tag="epst")
            nc.vector.memset(epst[:], EPS)

            # --- persistent activation tensors ---
            x = res.tile([128, DC, T], F32R, tag="x")          # residual, feature-major
            QT = res.tile([128, DC, T], BF16, tag="QT")
            KT = res.tile([128, DC, T], BF16, tag="KT")
            Vt = [res.tile([128, H, HS + 1], BF16, tag=f"V{t}", name=f"Vt{t}")
                  for t in range(TT)]

            # V ones-column (written once; V evacs only touch [:, :, 0:HS])
            for t in range(TT):
                nc.sync.dma_start(Vt[t][:, :, HS:HS + 1], onesv_d[:].unsqueeze(2))

            # ---------- embedding ----------
            idt = scr.tile([128, TT], I32, tag="ids")
            nc.sync.dma_start(idt[:], ids_d[:].rearrange("(tt p) -> p tt", p=128))
            for t in range(TT):
                xtok = scr.tile([128, D], F32, tag="xtok", bufs=2)
                nc.gpsimd.indirect_dma_start(
                    out=xtok[:], out_offset=None, in_=tok_d[:],
                    in_offset=bass.IndirectOffsetOnAxis(ap=idt[:, t:t + 1], axis=0))
                for dc in range(DC):
                    pt = ps.tile([128, 128], F32, tag="acc", bufs=2)
                    nc.tensor.transpose(pt[:], xtok[:, dc * 128:(dc + 1) * 128], ident[:])
                    pos = scr.tile([128, 128], F32, tag="pos", bufs=2)
                    nc.sync.dma_start(
                        pos[:], posT_d[dc * 128:(dc + 1) * 128, t * 128:(t + 1) * 128])
                    nc.vector.tensor_add(x[:, dc, t * 128:(t + 1) * 128], pt[:], pos[:])

            # ---------- layernorm helpers (stats woven into producer phases) ----------
            def ln_stats(J):
                """Compute mean/rstd broadcasts for x[:, :, J*512:(J+1)*512]."""
                sl = slice(J * 512, (J + 1) * 512)
                psx = ps.tile([1, 512], F32, tag="pss2", bufs=2)
                for dc in range(DC):
                    nc.tensor.matmul(psx[:], ones128f[:], x[:, dc, sl],
                                     start=(dc == 0), stop=(dc == DC - 1))
                psq = ps.tile([1, 512], F32, tag="pss2", bufs=2)
                for dc in range(DC):
                    xsq = scr.tile([128, 512], BF16, tag="w512b", bufs=2)
                    nc.vector.tensor_mul(xsq[:], x[:, dc, sl], x[:, dc, sl])
                    nc.tensor.matmul(psq[:], ones128b[:], xsq[:],
                                     start=(dc == 0), stop=(dc == DC - 1))
                sxr = scr.tile([1, 512], F32, tag="statJ", bufs=4)
                nc.scalar.activation(sxr[:], psx[:], AF.Identity, scale=1.0 / D)
                sq1 = scr.tile([1, 512], F32, tag="statJ", bufs=4)
                nc.vector.tensor_mul(sq1[:], sxr[:], sxr[:])
                svar = scr.tile([1, 512], F32, tag="statJ", bufs=4)
                with nc.allow_low_precision(reason="LN variance combine"):
                    nc.vector.scalar_tensor_tensor(
                        out=svar[:], in0=psq[:], scalar=1.0 / D, in1=sq1[:],
                        op0=OP.mult, op1=OP.subtract)
                lnv1 = scr.tile([1, 512], F32, tag="statJ", bufs=4)
                nc.scalar.activation(lnv1[:], svar[:], AF.Ln, bias=epst[:1, :1])
                rstd1 = scr.tile([1, 512], F32, tag="statJ", bufs=4)
                nc.scalar.activation(rstd1[:], lnv1[:], AF.Exp, scale=-0.5)
                mb = scr.tile([128, 512], F32, tag="lnmr", bufs=4)
                nc.gpsimd.partition_broadcast(mb[:], sxr[:])
                rstdb = scr.tile([128, 512], F32, tag="lnmr", bufs=4)
                nc.gpsimd.partition_broadcast(rstdb[:], rstd1[:])
                return mb, rstdb

            def ln_apply(J, stats, out_t):
                mb, rstdb = stats
                sl = slice(J * 512, (J + 1) * 512)
                for dc in range(DC):
                    t1 = scr.tile([128, 512], F32, tag="w512", bufs=4)
                    nc.vector.tensor_tensor(out=t1[:], in0=x[:, dc, sl], in1=mb[:],
                                            op=OP.subtract)
                    nc.vector.tensor_mul(out_t[:, dc, sl], t1[:], rstdb[:])

            # ---------- layers ----------
            nstats = [ln_stats(0), ln_stats(1)]
            for l in range(L):
                hO = res.tile([128, DC, T], BF16, tag="hO", bufs=1, name=f"h1_{l}")
                for J in range(2):
                    ln_apply(J, nstats[J], hO)

                qkb = scr.tile([128, 12], F32, tag="qkb", bufs=2)
                nc.sync.dma_start(qkb[:], qkb_d[l])

                # === Q, K projections (m 0..5 -> Q, 6..11 -> K) ===
                for m in [0, 6, 1, 7, 2, 8, 3, 9, 4, 10, 5, 11]:
                    wm = wts.tile([128, DC, 128], BF16, tag="wm", bufs=8)
                    nc.sync.dma_start(
                        wm[:], wqkv_d[l, m].rearrange("p (dc e) -> p dc e", dc=DC))
                    for J in range(2):
                        sl = slice(J * 512, (J + 1) * 512)
                        pq = ps.tile([128, 512], F32, tag="aux", bufs=2)
                        for dc in range(DC):
                            nc.tensor.matmul(pq[:], wm[:, dc, :], hO[:, dc, sl],
                                             start=(dc == 0), stop=(dc == DC - 1))
                        dst = QT if m < 6 else KT
                        nc.scalar.activation(dst[:, m % 6, sl], pq[:], AF.Identity,
                                             bias=qkb[:, m:m + 1])

                # === V projection (token-major, t-outer) ===
                vws = []
                for e2 in range(2):
                    vw = wts.tile([128, DC, 384], BF16, tag="vw", bufs=2,
                                  name=f"vw{e2}")
                    nc.sync.dma_start(
                        vw[:], wv_d[l, e2].rearrange("p (dc e) -> p dc e", dc=DC))
                    vws.append(vw)
                for t in range(TT):
                    for e2 in range(2):
                        pv = ps.tile([128, 384], F32, tag="acc", bufs=2)
                        for dc in range(DC):
                            nc.tensor.matmul(pv[:], hO[:, dc, t * 128:(t + 1) * 128],
                                             vws[e2][:, dc, :],
                                             start=(dc == 0), stop=(dc == DC - 1))
                        h0 = e2 * 6
                        nc.scalar.activation(
                            Vt[t][:, h0:h0 + 6, 0:HS],
                            pv[:].rearrange("p (h e) -> p h e", h=6), AF.Identity)

                # === attention: head pairs share one 2-bank psum, single exp ===
                def emit_S(hp, J):
                    ec = hp
                    nblk = 4 * J + 4
                    tiles = []   # per i: (at [128,2,512], c0)
                    for i in range(nblk):
                        r = i - 4 * J
                        c0 = max(r, 0) * 128
                        pss = ps.tile([128, 1024], F32, tag="pss2", bufs=2)
                        for sub in range(2):
                            po = sub * 64
                            o0 = sub * 512
                            nc.tensor.matmul(
                                pss[:, o0 + c0:o0 + 512],
                                KT[po:po + 64, ec, i * 128:(i + 1) * 128],
                                QT[po:po + 64, ec, J * 512 + c0:(J + 1) * 512],
                                start=True, stop=(r < 0))
                            if r >= 0:
                                nc.tensor.matmul(
                                    pss[:, o0 + c0:o0 + c0 + 128],
                                    ustair[:], vramp[:], start=False, stop=True)
                        at = scr.tile([128, 2, 512], BF16, tag="A", bufs=14)
                        nc.scalar.activation(
                            at[:, :, c0:512],
                            pss[:].rearrange("p (s c) -> p s c", s=2)[:, :, c0:512],
                            AF.Exp, scale=1.0 / 8)
                        tiles.append((at, c0))
                    return tiles

                def emit_AV(hp, J, tiles):
                    ec = hp
                    nblk = 4 * J + 4
                    for sub in range(2):
                        hh = 2 * hp + sub
                        po = sub * 64
                        pso = ps.tile([65, 512], F32,
                                      tag=("acc" if sub == 0 else "aux"), bufs=2)
                        for i in range(nblk):
                            at, c0 = tiles[i]
                            nc.tensor.matmul(pso[:, c0:512], Vt[i][:, hh, :],
                                             at[:, sub, c0:512],
                                             start=(i == 0), stop=(i == nblk - 1))
                        rec = scr.tile([1, 512], F32, tag="rec", bufs=2)
                        with nc.allow_low_precision(reason="softmax denom recip"):
                            nc.vector.reciprocal(rec[:], pso[64:65, :])
                        rb = scr.tile([64, 512], F32, tag="rb", bufs=2)
                        nc.gpsimd.partition_broadcast(rb[:], rec[:])
                        nc.vector.tensor_mul(hO[po:po + 64, ec, J * 512:(J + 1) * 512],
                                             pso[0:64, :], rb[:])

                # === attention: head pairs share one 2-bank psum, single exp ===
                def emit_S(hp, J):
                    ec = hp
                    nblk = 4 * J + 4
                    tiles = []
                    for i in range(nblk):
                        r = i - 4 * J
                        c0 = max(r, 0) * 128
                        pss = ps.tile([128, 1024], F32, tag="pss2", bufs=2)
                        for sub in range(2):
                            po = sub * 64
                            o0 = sub * 512
                            nc.tensor.matmul(
                                pss[:, o0 + c0:o0 + 512],
                                KT[po:po + 64, ec, i * 128:(i + 1) * 128],
                                QT[po:po + 64, ec, J * 512 + c0:(J + 1) * 512],
                                start=True, stop=(r < 0))
                            if r >= 0:
                                nc.tensor.matmul(
                                    pss[:, o0 + c0:o0 + c0 + 128],
                                    ustair[:], vramp[:], start=False, stop=True)
                        at = scr.tile([128, 2, 512], BF16, tag="A", bufs=14)
                        nc.scalar.activation(
                            at[:, :, c0:512],
                            pss[:].rearrange("p (s c) -> p s c", s=2)[:, :, c0:512],
                            AF.Exp, scale=1.0 / 8)
                        tiles.append((at, c0))
                    return tiles

                def emit_AV(hp, J, tiles):
                    ec = hp
                    nblk = 4 * J + 4
                    for sub in range(2):
                        hh = 2 * hp + sub
                        po = sub * 64
                        pso = ps.tile([65, 512], F32,
                                      tag=("acc" if sub == 0 else "aux"), bufs=2)
                        for i in range(nblk):
                            at, c0 = tiles[i]
                            nc.tensor.matmul(pso[:, c0:512], Vt[i][:, hh, :],
                                             at[:, sub, c0:512],
                                             start=(i == 0), stop=(i == nblk - 1))
                        rec = scr.tile([1, 512], F32, tag="rec", bufs=2)
                        with nc.allow_low_precision(reason="softmax denom recip"):
                            nc.vector.reciprocal(rec[:], pso[64:65, :])
                        rb = scr.tile([64, 512], F32, tag="rb", bufs=2)
                        nc.gpsimd.partition_broadcast(rb[:], rec[:])
                        nc.vector.tensor_mul(hO[po:po + 64, ec, J * 512:(J + 1) * 512],
                                             pso[0:64, :], rb[:])

                groups = [(hp, J) for J in range(2) for hp in range(6)]
                pend = None
                for gg in groups:
                    tiles = emit_S(*gg)
                    if pend is not None:
                        emit_AV(pend[0][0], pend[0][1], pend[1])
                    pend = (gg, tiles)
                emit_AV(pend[0][0], pend[0][1], pend[1])

                # === output projection + residual; LN2 stats woven in ===
                bocol = scr.tile([128, DC], F32, tag="bocol", bufs=2)
                nc.sync.dma_start(bocol[:], borow_d[l].rearrange(
                    "one (dc p) -> p (one dc)", p=128))
                wos = []
                for m in range(DC):
                    wot = wts.tile([128, DC, 128], BF16, tag="wm", bufs=8,
                                   name=f"wot{m}")
                    nc.sync.dma_start(
                        wot[:], wo_d[l, m].rearrange("p (dc e) -> p dc e", dc=DC))
                    wos.append(wot)
                for J in range(2):
                    sl = slice(J * 512, (J + 1) * 512)
                    for m in range(DC):
                        p2 = ps.tile([128, 512], F32, tag="aux", bufs=2)
                        for dc in range(DC):
                            nc.tensor.matmul(p2[:], wos[m][:, dc, :], hO[:, dc, sl],
                                             start=(dc == 0), stop=(dc == DC - 1))
                        nc.vector.scalar_tensor_tensor(
                            out=x[:, m, sl], in0=p2[:], scalar=bocol[:, m:m + 1],
                            in1=x[:, m, sl], op0=OP.add, op1=OP.add)
                    nstats[J] = ln_stats(J)

                h2 = res.tile([128, DC, T], BF16, tag="hO", bufs=1, name=f"h2_{l}")
                # === FFN: FFN1 streams, FFN2 m-outer over resident gf/w2 ===
                b1c = scr.tile([128, FC], F32, tag="b1c", bufs=2)
                nc.sync.dma_start(b1c[:], b1c_d[l])
                b2col = scr.tile([128, DC], F32, tag="b2col", bufs=2)
                nc.sync.dma_start(b2col[:], b2row_d[l].rearrange(
                    "one (dc p) -> p (one dc)", p=128))
                for J in range(2):
                    sl = slice(J * 512, (J + 1) * 512)
                    ln_apply(J, nstats[J], h2)
                    gfs = []
                    for fc in range(FC):
                        w1t = wts.tile([128, DC, 128], BF16, tag="w1", bufs=3)
                        nc.sync.dma_start(
                            w1t[:], w1_d[l, fc].rearrange("p (dc e) -> p dc e", dc=DC))
                        pf = ps.tile([128, 512], F32, tag="aux", bufs=2)
                        for dc in range(DC):
                            nc.tensor.matmul(pf[:], w1t[:, dc, :], h2[:, dc, sl],
                                             start=(dc == 0), stop=(dc == DC - 1))
                        gf = scr.tile([128, 512], BF16, tag="gf", bufs=FC + 1)
                        nc.scalar.activation(gf[:], pf[:], AF.Gelu,
                                             bias=b1c[:, fc:fc + 1])
                        gfs.append(gf)
                    for m in range(DC):
                        w2m = wts.tile([128, FC, 128], BF16, tag="w2m", bufs=3)
                        nc.scalar.dma_start(
                            w2m[:], w2_d[l, m].rearrange("p (fc e) -> p fc e", fc=FC))
                        p2 = ps.tile([128, 512], F32, tag="acc", bufs=2)
                        for fc in range(FC):
                            nc.tensor.matmul(p2[:], w2m[:, fc, :],
                                             gfs[fc][:], start=(fc == 0),
                                             stop=(fc == FC - 1))
                        nc.vector.scalar_tensor_tensor(
                            out=x[:, m, sl], in0=p2[:], scalar=b2col[:, m:m + 1],
                            in1=x[:, m, sl], op0=OP.add, op1=OP.add)
                    nstats[J] = ln_stats(J)

            # ---------- final LN + mean pool + classifier ----------
            hf = res.tile([128, DC, T], BF16, tag="hO", bufs=1, name="hf")
            for J in range(2):
                ln_apply(J, nstats[J], hf)
            wct = scr.tile([128, DC, NCLS], BF16, tag="wct")
            nc.sync.dma_start(wct[:], wc_d[:].rearrange("(dc p) n -> p dc n", p=128))
            bcr = scr.tile([1, NCLS], BF16, tag="bcr")
            nc.sync.dma_start(bcr[:], bcrow_d[:])
            pooled = []
            for dc in range(DC):
                red = scr.tile([128, 1], F32, tag="red", bufs=DC)
                nc.vector.tensor_reduce(red[:], hf[:, dc, :], axis=mybir.AxisListType.X,
                                        op=OP.add)
                pr = scr.tile([128, 1], BF16, tag="pooled", bufs=DC)
                nc.scalar.activation(pr[:], red[:], AF.Identity, scale=1.0 / T)
                pooled.append(pr)
            pcls = ps.tile([1, NCLS], F32, tag="acc", bufs=2)
            nc.tensor.matmul(pcls[:], onescol[:, :1], bcr[:], start=True, stop=False)
            for dc in range(DC):
                nc.tensor.matmul(pcls[:], pooled[dc][:], wct[:, dc, :],
                                 start=False, stop=(dc == DC - 1))
            ob = scr.tile([1, NCLS], F32, tag="ob")
            nc.vector.tensor_copy(ob[:], pcls[:])
            nc.sync.dma_start(out_d[:], ob[:])

    nc.compile()
    return nc


def _pack_params(params):
    import ml_dtypes

    BF = ml_dtypes.bfloat16
    g = {k: np.asarray(v, dtype=np.float32) for k, v in params.items()}

    # ---- fold LN gains/biases into consumer projections ----
    # device LN emits (x-m)*rstd; true h = g*that + b, so for consumer
    # y = W^T h + c:  W' = diag(g) @ W,  c' = c + W^T b.
    wq = g["wq"].transpose(0, 2, 1, 3).reshape(L, D, D)
    wk = g["wk"].transpose(0, 2, 1, 3).reshape(L, D, D)
    wv = g["wv"].transpose(0, 2, 1, 3).reshape(L, D, D)
    bq = g["bq"].reshape(L, D).copy()
    bk = g["bk"].reshape(L, D).copy()
    bv = g["bv"].reshape(L, D).copy()
    w1 = g["w1"].copy()
    b1 = g["b1"].copy()
    wc = g["wc"].copy()
    bc = g["bc"].copy()
    bo = g["bo"].copy()
    wo = g["wo"]

    for l in range(L):
        g1, bql = g["ln1_g"][l], g["ln1_b"][l]
        bq[l] += wq[l].T @ bql
        bk[l] += wk[l].T @ bql
        bv[l] += wv[l].T @ bql
        wq[l] = wq[l] * g1[:, None]
        wk[l] = wk[l] * g1[:, None]
        wv[l] = wv[l] * g1[:, None]
        g2, b2l = g["ln2_g"][l], g["ln2_b"][l]
        b1[l] += w1[l].T @ b2l
        w1[l] = w1[l] * g2[:, None]
        # V bias folds through attention into the output projection bias:
        # O = raw/den + bv  =>  bo' = bo + wo^T bv
        bo[l] += wo[l].T @ bv[l]
    bc += wc.T @ g["lnf_b"]
    wc = wc * g["lnf_g"][:, None]

    wqk = np.concatenate([wq, wk], axis=2)                       # [L, D, 2D]

    # packed lhsT tiles: tile[l, m, p, dc*128+e] = W[l, dc*128+p, m*128+e]
    def pack_lhsT(w, mtiles):
        return np.ascontiguousarray(
            w.reshape(L, DC, 128, mtiles, 128).transpose(0, 3, 2, 1, 4)
            .reshape(L, mtiles, 128, DC * 128)).astype(BF)

    wqkv_p = pack_lhsT(wqk, 12)
    # V weight (moving operand): tile[l, e2, p, dc*384+e] = wv[l, dc*128+p, e2*384+e]
    wv_p = np.ascontiguousarray(
        wv.reshape(L, DC, 128, 2, 384).transpose(0, 3, 2, 1, 4)
        .reshape(L, 2, 128, DC * 384)).astype(BF)
    wo_p = pack_lhsT(wo, DC)
    w1_p = pack_lhsT(w1, FC)
    # w2 per-m lhsT tiles: w2m[l, m, p, fc*128+e] = w2[l, fc*128+p, m*128+e]
    w2_p = np.ascontiguousarray(
        g["w2"].reshape(L, FC, 128, DC, 128).transpose(0, 3, 2, 1, 4)
        .reshape(L, DC, 128, FC * 128)).astype(BF)

    qkb = np.empty((L, 128, 12), np.float32)
    for l in range(L):
        qk = np.concatenate([bq[l], bk[l]])
        for m in range(12):
            qkb[l, :, m] = qk[m * 128:(m + 1) * 128]

    b1c = np.empty((L, 128, FC), np.float32)
    for l in range(L):
        for fc in range(FC):
            b1c[l, :, fc] = b1[l][fc * 128:(fc + 1) * 128]

    # causal-mask generators over the diagonal 128-block triangle:
    # mask[s, j] = sum_c U[c,s]*Vr[c,j] = BIG * max(0, s - j)
    cc = np.arange(128)
    ustair = (cc[None, :] >= cc[:, None]).astype(BF)
    vramp = np.where(cc[None, :] < cc[:, None], np.float32(BIG), 0.0).astype(BF)

    return {
        "tok_emb": g["tok_emb"],
        "posT": np.ascontiguousarray(g["pos_emb"][:T].T),
        "wqkv": wqkv_p,
        "wv": wv_p,
        "wo": wo_p,
        "w1": w1_p,
        "w2": w2_p,
        "wc": wc.astype(BF),
        "qkb": qkb,
        "b1c": b1c,
        "borow": bo.reshape(L, 1, D),
        "b2row": g["b2"].reshape(L, 1, D),
        "bcrow": bc.reshape(1, NCLS).astype(BF),
        "ident": np.eye(128, dtype=np.float32),
        "ustair": ustair,
        "vramp": vramp,
        "onescol": np.ones((1, 128), BF),
        "onesrow": np.ones((1, 512), BF),
        "ones128f": np.ones((128, 1), np.float32),
        "ones128b": np.ones((128, 1), BF),
        "onesv": np.ones((128, H), BF),
    }


def kernel(token_ids, params):
    from concourse.bass_utils import run_bass_kernel_spmd

    if "nc" not in _CACHE:
        _CACHE["nc"] = _build()
    nc = _CACHE["nc"]

    common = _pack_params(params)
    ids32 = np.asarray(token_ids).astype(np.int32)
    in_maps = [{**common, "ids": ids32[c]} for c in range(B)]
    res = run_bass_kernel_spmd(nc, in_maps, list(range(B)))
    out = np.stack([res.results[c]["out"][0] for c in range(B)], axis=0)
    return out.astype(np.float32)
